# revision 27
# baseline (speedup 1.0000x reference)
"""Trainium2 Bass kernel for LLFullObjectCondensation loss (N=80000, K=512, C=2).

Strategy (8 NeuronCores, data-parallel over hits):
  - Each core gets a 10000-hit shard (padded to 79*128=10112), laid out [128, 79].
  - P1: per-hit quantities (q, payload, weights) as full-width [128,79] ops.
  - P2: per-object max of (beta+1)-weighted one-hot tiles (DVE build + running
        max), 4 PE transposes + reductions -> Bloc, packed to row BlocF [1,K].
  - AllReduce-max of BlocF; P3 selection pass overlaps: Isel = (bm == BlocB),
        PE matmul with sel3 stationary -> selPT [3,K] rows, gated by
        keepF = (BlocF == BglobF), AllReduce-add -> global x_alpha/q_alpha rows.
  - P5 loop 1 (software-pipelined): d2 via PE matmul (contract-4 trick), sqrt
        on ACT, min on DVE, repulsion row-sums via PE, self-distance gather via
        DVE row-reduce with the weighted one-hot.
  - P5 loop 2: segment sums as PE matmuls, rhs_seg [128,6] stationary, bm
        moving -> segPT [6,K] rows.
  - AllReduce-add of all per-object rows, then row-layout [1,K] assembly of
        the scalar loss.
All one-hot builds and elementwise work run on DVE/ACT (GPSIMD's software
tensor_scalar measured ~8us per [128,512] tile vs DVE's ~0.55us).
"""
import sys
import numpy as np

for _p in ("/opt/trn_rl_repo", "/root/.axon_site/_ro/trn_rl_repo"):
    if _p not in sys.path:
        sys.path.append(_p)

N = 80000
K = 512
NCORES = 8
S = N // NCORES          # 10000 hits per core
P = 128
T = 79                   # tiles per core, T*P = 10112 >= S
SP = T * P
KB = K // P              # 4 k-blocks
EPS = 1e-9
SQ_BIAS = 2e-2           # reference uses 1e-6; extra margin absorbs fp32r
                         # matmul rounding of the expanded |x|^2-2x.a+|a|^2
                         # form so sqrt never sees a negative input (measured:
                         # 1e-3 still went NaN -> fp32r product error ~5e-3 abs
                         # on O(20) terms; the 0.02 bias shifts the hinge by
                         # ~1.2e-3 relative on the total loss, gate is 2e-2)

_CACHE = {}


def _build(cc_mode='all'):
    import concourse.bass as bass
    import concourse.bacc as bacc
    import concourse.mybir as mybir
    import concourse.tile as tile
    from concourse import masks

    f32 = mybir.dt.float32
    f32r = mybir.dt.float32r
    i32 = mybir.dt.int32
    u16 = mybir.dt.uint16
    f16 = mybir.dt.float16
    AF = mybir.ActivationFunctionType
    OP = mybir.AluOpType

    nc = bacc.Bacc("TRN2", target_bir_lowering=False, debug=False,
                   num_devices=NCORES)

    di = {}
    def din(name, shape):
        di[name] = nc.dram_tensor(name, shape, f32, kind="ExternalInput")
        return di[name]

    din("beta_r", [P, T])
    din("cc", [P, T, 2])
    din("pE", [P, T])
    din("ppos", [P, T, 2])
    din("ptime", [P, T])
    din("pid", [P, T, 6])
    din("tE", [P, T])
    din("tpos", [P, T, 2])
    din("ttime", [P, T])
    din("tidx", [P, T])
    din("valid", [P, T])
    out_d = nc.dram_tensor("out", [1, 1], f32, kind="ExternalOutput")

    with tile.TileContext(nc) as tc:
        with (
            tc.tile_pool(name="const", bufs=1) as cpool,
            tc.tile_pool(name="io", bufs=1) as io,
            tc.tile_pool(name="dram", bufs=1, space="DRAM") as dram,
            tc.tile_pool(name="psA", bufs=2, space="PSUM") as psA,
            tc.tile_pool(name="acc", bufs=1, space="PSUM") as accp,
        ):
            # ---------- constants ----------
            ident = cpool.tile([P, P], f32)
            masks.make_identity(nc, ident[:])
            iotaI = cpool.tile([P, K], i32)
            nc.gpsimd.iota(iotaI[:], pattern=[[1, K]], base=0,
                           channel_multiplier=0)
            iotaF = cpool.tile([P, K], f32)
            nc.vector.tensor_copy(iotaF[:], iotaI[:])
            iotaH = cpool.tile([P, K], f16)
            nc.vector.tensor_copy(iotaH[:], iotaI[:])
            onescol = cpool.tile([P, 1], f32)
            nc.vector.memset(onescol[:], 1.0)
            onesrow = cpool.tile([1, P], f32)
            nc.vector.memset(onesrow[:], 1.0)

            _cb = {}
            def cbias(val):
                """[128,1] constant column for activation bias operands."""
                if val not in _cb:
                    ct = cpool.tile([P, 1], f32, name=f"cb{len(_cb)}")
                    nc.vector.memset(ct[:], val)
                    _cb[val] = ct
                return _cb[val][:]

            # ---------- load inputs ----------
            sb = {}
            for name, h in di.items():
                t_sb = io.tile(list(h.shape), f32, name=f"sb_{name}")
                nc.sync.dma_start(t_sb[:], h.ap())
                sb[name] = t_sb

            # ---------- P1: per-hit prep (all [128,T]-wide ops) ----------
            V = nc.vector
            SC = nc.scalar

            def wtile(name, shape=None, dtype=None):
                return io.tile(shape or [P, T], dtype or f32, name=name)
            u8 = mybir.dt.uint8

            beta = wtile("beta")
            V.tensor_scalar(beta[:], sb["beta_r"][:], 1e-6, 1.0 - 1e-6,
                            OP.max, OP.min)
            betap1 = wtile("betap1")
            SC.activation(betap1[:], beta[:], AF.Identity, bias=cbias(1.0))
            rb1 = wtile("rb1")
            V.reciprocal(rb1[:], betap1[:])
            onem = wtile("onem")
            SC.activation(onem[:], beta[:], AF.Identity, bias=cbias(1.0), scale=-1.0)
            recm = wtile("recm")
            V.reciprocal(recm[:], onem[:])
            ratio = wtile("ratio")
            V.tensor_tensor(ratio[:], betap1[:], recm[:], OP.mult)

            is_noise = wtile("is_noise")
            V.tensor_scalar(is_noise[:], sb["tidx"][:], -1.0, None, OP.is_equal)
            is_obj = wtile("is_obj")
            V.tensor_scalar(is_obj[:], sb["tidx"][:], 0.0, None, OP.is_ge)

            # energy weights w = relu(min(wr,1)) ; wr=(tE-0.5)/9.5
            wr = wtile("wr")
            SC.activation(wr[:], sb["tE"][:], AF.Identity, bias=cbias(-0.5 / 9.5),
                          scale=1.0 / 9.5)
            ew = wtile("ew")
            V.tensor_scalar(ew[:], wr[:], 1.0, 0.0, OP.min, OP.max)
            pw = wtile("pw")
            V.tensor_tensor(pw[:], beta[:], ew[:], OP.mult)
            V.tensor_tensor(pw[:], pw[:], is_obj[:], OP.mult)

            # --- energy loss pieces (pre-transcendental) ---
            ediff_r = wtile("ediff_r")
            V.tensor_tensor(ediff_r[:], sb["tE"][:], sb["pE"][:], OP.subtract)
            ediff = wtile("ediff")
            SC.activation(ediff[:], ediff_r[:], AF.Abs)
            ed2 = wtile("ed2")
            V.tensor_tensor(ed2[:], ediff[:], ediff[:], OP.mult)
            ed001 = wtile("ed001")
            SC.activation(ed001[:], ediff[:], AF.Copy, scale=0.001)

            # --- position loss pieces ---
            dpos = wtile("dpos", [P, T, 2])
            V.tensor_tensor(dpos[:], sb["tpos"][:], sb["ppos"][:], OP.subtract)
            V.tensor_tensor(dpos[:], dpos[:], dpos[:], OP.mult)
            d2p = wtile("d2p")
            V.tensor_tensor(d2p[:], dpos[:, :, 0], dpos[:, :, 1], OP.add)

            # --- timing loss pieces ---
            dtm = wtile("dtm")
            V.tensor_tensor(dtm[:], sb["ttime"][:], sb["ptime"][:], OP.subtract)
            adt = wtile("adt")
            SC.activation(adt[:], dtm[:], AF.Abs)
            dt2 = wtile("dt2")
            V.tensor_tensor(dt2[:], dtm[:], dtm[:], OP.mult)
            lint = wtile("lint")
            SC.activation(lint[:], adt[:], AF.Identity, bias=cbias(-4.0), scale=4.0)
            ltt = wtile("ltt", dtype=u8)
            V.tensor_scalar(ltt[:], adt[:], 2.0, None, OP.is_lt)
            ht = wtile("ht")
            V.select(ht[:], ltt[:], dt2[:], lint[:])
            yt = wtile("yt")
            SC.activation(yt[:], ht[:], AF.Copy, scale=1.0 / 6.0)

            # --- classification loss ---
            pid2 = wtile("pid2", [P, T, 6])
            V.tensor_tensor(pid2[:], sb["pid"][:], sb["pid"][:], OP.mult)
            cred = wtile("cred")
            V.tensor_reduce(cred[:], pid2[:], mybir.AxisListType.X, OP.add)

            # --- transcendental block: Exp, then Sqrt, then Ln (grouped to
            # limit ACT table swaps) ---
            ex = wtile("ex")
            SC.activation(ex[:], ed2[:], AF.Exp, scale=-0.1)
            xp = wtile("xp")
            SC.activation(xp[:], d2p[:], AF.Sqrt, bias=cbias(0.01), scale=0.01)

            lnr = wtile("lnr")
            SC.activation(lnr[:], ratio[:], AF.Ln)
            # q = (0.5*ln(ratio))^2 + 0.1, zeroed on padding
            halfln = wtile("halfln")
            SC.activation(halfln[:], lnr[:], AF.Copy, scale=0.5)
            q = wtile("q")
            V.tensor_tensor(q[:], halfln[:], halfln[:], OP.mult)
            V.scalar_tensor_tensor(q[:], q[:], 0.1, sb["valid"][:],
                                   OP.add, OP.mult)

            # energy softclip
            ye = wtile("ye")
            V.tensor_tensor(ye[:], ex[:], ed001[:], OP.add)
            lnye = wtile("lnye")
            SC.activation(lnye[:], ye[:], AF.Ln, bias=cbias(1.0))
            gte = wtile("gte", dtype=u8)
            V.tensor_scalar(gte[:], ye[:], 1.0, None, OP.is_gt)
            esc = wtile("esc")
            V.select(esc[:], gte[:], lnye[:], ye[:])

            # position huber + softclip
            xp2 = wtile("xp2")
            V.tensor_tensor(xp2[:], xp[:], xp[:], OP.mult)
            linp = wtile("linp")
            SC.activation(linp[:], xp[:], AF.Identity, bias=cbias(-100.0), scale=20.0)
            ltp = wtile("ltp", dtype=u8)
            V.tensor_scalar(ltp[:], xp[:], 10.0, None, OP.is_lt)
            hp = wtile("hp")
            V.select(hp[:], ltp[:], xp2[:], linp[:])
            yp = wtile("yp")
            SC.activation(yp[:], hp[:], AF.Copy, scale=1.0 / 3.0)
            lnyp = wtile("lnyp")
            SC.activation(lnyp[:], yp[:], AF.Ln, bias=cbias(1.0))
            gtp = wtile("gtp", dtype=u8)
            V.tensor_scalar(gtp[:], yp[:], 1.0, None, OP.is_gt)
            psc = wtile("psc")
            V.select(psc[:], gtp[:], lnyp[:], yp[:])

            # timing softclip
            lnyt = wtile("lnyt")
            SC.activation(lnyt[:], yt[:], AF.Ln, bias=cbias(1.0))
            gtt = wtile("gtt", dtype=u8)
            V.tensor_scalar(gtt[:], yt[:], 1.0, None, OP.is_gt)
            tsc = wtile("tsc")
            V.select(tsc[:], gtt[:], lnyt[:], yt[:])

            # payload = 10*esc + 3*psc + 6*tsc + (1e-8/6)*cred
            esc10 = wtile("esc10")
            SC.activation(esc10[:], esc[:], AF.Copy, scale=10.0)
            pay = wtile("pay")
            V.scalar_tensor_tensor(pay[:], psc[:], 3.0, esc10[:],
                                   OP.mult, OP.add)
            V.scalar_tensor_tensor(pay[:], tsc[:], 6.0, pay[:],
                                   OP.mult, OP.add)
            V.scalar_tensor_tensor(pay[:], cred[:], 1e-8 / 6.0, pay[:],
                                   OP.mult, OP.add)
            paypw = wtile("paypw")
            V.tensor_tensor(paypw[:], pay[:], pw[:], OP.mult)

            # selection rhs: [x0, x1, q] (stationary operand for P3 matmuls)
            sel3 = wtile("sel3", [P, T, 3], dtype=f32r)
            SC.activation(sel3[:, :, 0:2], sb["cc"][:], AF.Copy)
            V.tensor_copy(sel3[:, :, 2], q[:])

            # d2-matmul lhsT quantities [-2x0, -2x1, 1, |x|^2] packed [P,T,4]
            prep4 = wtile("prep4", [P, T, 4])
            SC.activation(prep4[:, :, 0:2], sb["cc"][:], AF.Copy, scale=-2.0)
            V.memset(prep4[:, :, 2], 1.0)
            ccsq = wtile("ccsq", [P, T, 2])
            V.tensor_tensor(ccsq[:], sb["cc"][:], sb["cc"][:], OP.mult)
            V.tensor_tensor(prep4[:, :, 3], ccsq[:, :, 0], ccsq[:, :, 1],
                            OP.add)

            # extras: [noise*beta, noise, |x|^2, q] free-reduced to [P,4],
            # then partition-reduced to a [1,4] row via PE (ready for AR3)
            extras = io.tile([P, 4], f32, name="extras")
            nb_t = wtile("nb_t")
            V.tensor_tensor(nb_t[:], is_noise[:], beta[:], OP.mult)
            V.tensor_reduce(extras[:, 0:1], nb_t[:], mybir.AxisListType.X, OP.add)
            V.tensor_reduce(extras[:, 1:2], is_noise[:], mybir.AxisListType.X, OP.add)
            V.tensor_reduce(extras[:, 2:3], prep4[:, :, 3], mybir.AxisListType.X, OP.add)
            V.tensor_reduce(extras[:, 3:4], q[:], mybir.AxisListType.X, OP.add)
            extrasF = io.tile([1, 4], f32, name="extrasF")
            with tc.tile_pool(name="exp", bufs=1, space="PSUM") as exp_p:
                exPS = exp_p.tile([1, 4], f32, name="exPS")
                nc.tensor.matmul(exPS[:], onescol[:], extras[:],
                                 start=True, stop=True)
                SC.activation(extrasF[:], exPS[:], AF.Copy)

            # transpose prep4 -> lhsT4r [4, T, 128] (f32r, rounded at the
            # ACT evacuation so the fp32r d2 matmul accepts it)
            lhsT4r = io.tile([4, T, P], f32r, name="lhsT4r")
            for r in range(4):
                tp = psA.tile([P, P], f32, name="tpose4", tag="tpose")
                nc.tensor.transpose(tp[0:T, :], prep4[:, :, r], ident[:])
                stage = io.tile([T, P], f32r, name=f"tstage{r}")
                SC.activation(stage[:], tp[0:T, :], AF.Copy)
                nc.sync.dma_start(lhsT4r[r:r + 1, :, :], stage[:])

            # ---------- P2: local per-object max of (beta+1)-weighted one-hot
            # (DVE builds; bm[p,k] = (iota[k]==tidx[p,t]) * (beta[p,t]+1)) ----
            runmax = io.tile([P, K], f32, name="runmax")
            V.memset(runmax[:], 0.0)
            with tc.tile_pool(name="bmp", bufs=3) as bmp:
                for t in range(T):
                    bm = bmp.tile([P, K], f32, name="bm")
                    V.tensor_scalar(bm[:], iotaF[:], sb["tidx"][:, t:t + 1],
                                    betap1[:, t:t + 1], OP.is_equal, OP.mult)
                    V.tensor_tensor(runmax[:], runmax[:], bm[:], OP.max)

            # partition-reduce runmax -> Bloc [128,4] (k = 128*b + p)
            Bloc = io.tile([P, KB], f32, name="Bloc")
            for b in range(KB):
                tp = psA.tile([P, P], f32, name="tpose", tag="tpose")
                nc.tensor.transpose(tp[:], runmax[:, b * P:(b + 1) * P], ident[:])
                V.reduce_max(Bloc[:, b:b + 1], tp[:], axis=mybir.AxisListType.X)

            # row layout: BlocF[0, 128*b+p] = Bloc[p, b]
            BlocF = io.tile([1, K], f32, name="BlocF")
            for b in range(KB):
                nc.sync.dma_start(BlocF[0:1, b * P:(b + 1) * P], Bloc[:, b:b + 1])

            # ---------- P4a: AllReduce-max of BlocF (overlaps with P3) -------
            arm_in = dram.tile([1, K], f32, name="arm_in")
            arm_out = dram.tile([1, K], f32, name="arm_out", addr_space="Shared")
            nc.sync.dma_start(arm_in[0:1, :], BlocF[:])
            if cc_mode in ('all', 'first', 'two'):
                nc.gpsimd.collective_compute(
                    "AllReduce", OP.max,
                    replica_groups=[list(range(NCORES))],
                    ins=[arm_in[:]], outs=[arm_out[:]],
                )
            else:
                nc.sync.dma_start(arm_out[:], arm_in[:])
            BglobF = io.tile([1, K], f32, name="BglobF")
            nc.sync.dma_start(BglobF[:], arm_out[0:1, :])

            # broadcast BlocF across partitions via PE: ones[1,P].T @ BlocF
            BlocB = io.tile([P, K], f32, name="BlocB")
            with tc.tile_pool(name="bcp", bufs=1, space="PSUM") as bcp:
                blocps = bcp.tile([P, K], f32, name="blocps")
                nc.tensor.matmul(blocps[:], onesrow[:], BlocF[:],
                                 start=True, stop=True)
                SC.activation(BlocB[:], blocps[:], AF.Copy)

            # ---------- P3: selection segment-sums -> selPT rows [3, K] ------
            with (
                tc.tile_pool(name="selpp", bufs=1, space="PSUM") as selpp,
                tc.tile_pool(name="bmp3", bufs=3) as bmp3,
            ):
                selPT = selpp.tile([3, K], f32, name="selPT")
                V.memset(selPT[:], 0.0)
                for t in range(T):
                    bm = bmp3.tile([P, K], f32, name="bm3")
                    V.tensor_scalar(bm[:], iotaF[:], sb["tidx"][:, t:t + 1],
                                    betap1[:, t:t + 1], OP.is_equal, OP.mult)
                    Isel = bmp3.tile([P, K], f32r, name="Isel")
                    V.tensor_tensor(Isel[:], bm[:], BlocB[:], OP.is_equal)
                    nc.tensor.matmul(selPT[:], sel3[:, t, :], Isel[:],
                                     start=False, stop=(t == T - 1),
                                     skip_group_check=True)
                selsbT = io.tile([3, K], f32, name="selsbT")
                SC.activation(selsbT[:], selPT[:], AF.Copy)

            # gate by global-winner mask and AllReduce-add.  Compute engines
            # must start at partition 0/32/64/96, so replicate keepF to 3
            # partitions via DMA and gate with one [3,K] multiply.
            keepF = io.tile([1, K], f32, name="keepF")
            V.tensor_tensor(keepF[:], BlocF[:], BglobF[:], OP.is_equal)
            keep3 = io.tile([3, K], f32, name="keep3")
            for r in range(3):
                nc.sync.dma_start(keep3[r:r + 1, :], keepF[:])
            sel_cT = io.tile([3, K], f32, name="sel_cT")
            V.tensor_tensor(sel_cT[:], selsbT[:], keep3[:], OP.mult)
            ar2_in = dram.tile([1, 3 * K], f32, name="ar2_in")
            ar2_out = dram.tile([1, 3 * K], f32, name="ar2_out",
                                addr_space="Shared")
            nc.sync.dma_start(ar2_in[0:1, :], sel_cT[:])   # row-major pack
            if cc_mode in ('all', 'two'):
                nc.gpsimd.collective_compute(
                    "AllReduce", OP.add,
                    replica_groups=[list(range(NCORES))],
                    ins=[ar2_in[:]], outs=[ar2_out[:]],
                )
            else:
                nc.sync.dma_start(ar2_out[:], ar2_in[:])

            # prebuild the first NPRE segment one-hots while AR2 is in
            # flight (issued before any AR2-dependent DVE op so the in-order
            # DVE queue can drain them during the collective)
            NPRE = 32
            bm6pre = io.tile([P, NPRE, K], f16, name="bm6pre")
            for t in range(NPRE):
                V.tensor_scalar(bm6pre[:, t, :], iotaH[:],
                                sb["tidx"][:, t:t + 1],
                                betap1[:, t:t + 1], OP.is_equal, OP.mult)

            # rhsD2 rows: [xa0; xa1; |xa|^2; 1].  |xa|^2 is computed on
            # partition 0 (xa0F/xa1F row tiles) and DMA'd into row 2.
            rhsD2 = io.tile([4, K], f32, name="rhsD2")
            V.memset(rhsD2[:], 1.0)
            nc.sync.dma_start(
                rhsD2[0:2, :],
                ar2_out[0:1, 0:2 * K].rearrange("o (r k) -> (o r) k", r=2))
            xa0F = io.tile([1, K], f32, name="xa0F")
            nc.sync.dma_start(xa0F[:], ar2_out[0:1, 0:K])
            xa1F = io.tile([1, K], f32, name="xa1F")
            nc.sync.dma_start(xa1F[:], ar2_out[0:1, K:2 * K])
            qaF = io.tile([1, K], f32, name="qaF")
            nc.sync.dma_start(qaF[:], ar2_out[0:1, 2 * K:3 * K])
            xsqF = io.tile([1, K], f32, name="xsqF")
            xsq_t = io.tile([1, K], f32, name="xsq_t")
            V.tensor_tensor(xsq_t[:], xa1F[:], xa1F[:], OP.mult)
            V.tensor_tensor(xsqF[:], xa0F[:], xa0F[:], OP.mult)
            V.tensor_tensor(xsqF[:], xsqF[:], xsq_t[:], OP.add)
            nc.sync.dma_start(rhsD2[2:3, :], xsqF[:])
            rhsD2r = io.tile([4, K], f32r, name="rhsD2r")
            V.tensor_copy(rhsD2r[:], rhsD2[:])

            # column-layout copies of q_alpha and beta_alpha for P7
            # (transposed while loop 1 runs; PE is only ~50% busy there)
            qb_rows = io.tile([2, K], f32, name="qb_rows")
            nc.sync.dma_start(qb_rows[0:1, :], qaF[:])
            nc.sync.dma_start(qb_rows[1:2, :], BglobF[:])
            qab2 = io.tile([P, KB, 2], f32, name="qab2")
            for b in range(KB):
                tp = psA.tile([P, P], f32, name="tpqb", tag="tpose")
                nc.tensor.transpose(tp[:, 0:2], qb_rows[0:2, b * P:(b + 1) * P],
                                    ident[0:2, 0:2])
                SC.activation(qab2[:, b, :], tp[:, 0:2], AF.Copy)

            qr = wtile("qr", dtype=f32r)      # rounded copy for fp32r matmul
            V.tensor_copy(qr[:], q[:])

            # ---------- P5 loop 1: d2 block, rep row-sums, self-distance -----
            # software-pipelined by one stage: d2 matmul for t+1 issues before
            # the rep matmul for t so the PE never blocks behind the sqrt.
            # rep accumulates q * relu(1 - s) directly (hinge on ACT).
            gstD = io.tile([P, T], f32, name="gstD")   # (beta+1)*s_self
            repP = accp.tile([1, K], f32, name="repP")
            V.memset(repP[:], 0.0)
            scr = io.tile([P, K], f16, name="scr")         # ttr full-out scratch
            with (
                tc.tile_pool(name="d2pool", bufs=3, space="PSUM") as d2pool,
                tc.tile_pool(name="sp", bufs=3) as sp,
                tc.tile_pool(name="bmp5", bufs=3) as bmp5,
            ):
                d2tiles = {}
                def d2mm(t):
                    d2ps = d2pool.tile([P, K], f32, name="d2ps")
                    nc.tensor.matmul(d2ps[:], lhsT4r[0:4, t, :], rhsD2r[:],
                                     start=True, stop=True)
                    d2tiles[t] = d2ps
                d2mm(0)
                for t in range(T):
                    if t + 1 < T:
                        d2mm(t + 1)
                    d2ps = d2tiles.pop(t)
                    bm = bmp5.tile([P, K], f16, name="bm5")
                    V.tensor_scalar(bm[:], iotaH[:], sb["tidx"][:, t:t + 1],
                                    betap1[:, t:t + 1], OP.is_equal, OP.mult)
                    sS = sp.tile([P, K], f16, name="sS")
                    SC.activation(sS[:], d2ps[:], AF.Sqrt, bias=cbias(SQ_BIAS))
                    rlu = sp.tile([P, K], f32r, name="rlu")
                    SC.activation(rlu[:], sS[:], AF.Relu, bias=cbias(1.0),
                                  scale=-1.0)
                    nc.tensor.matmul(repP[:], qr[:, t:t + 1], rlu[:],
                                     start=False, stop=(t == T - 1),
                                     skip_group_check=True)
                    V.scalar_tensor_tensor(
                        scr[:], bm[:], 1.0, sS[:], OP.bypass, OP.mult,
                        accum_out=gstD[:, t:t + 1])

            # ---------- global per-hit math for segment rhs ----------
            qrb = wtile("qrb")
            V.tensor_tensor(qrb[:], q[:], rb1[:], OP.mult)
            sself = wtile("sself")              # sqrt(d2_self + SQ_BIAS)
            V.tensor_tensor(sself[:], gstD[:], rb1[:], OP.mult)
            G2 = wtile("G2")                    # d2_self
            V.tensor_tensor(G2[:], sself[:], sself[:], OP.mult)
            V.tensor_scalar(G2[:], G2[:], SQ_BIAS, None, OP.subtract)
            s2 = wtile("s2")                    # min(s_self, 1)
            V.tensor_scalar(s2[:], sself[:], 1.0, None, OP.min)
            rhs_seg = io.tile([P, T, 6], f16, name="rhs_seg")
            # att' = q*d2_self/(b+1)
            V.tensor_tensor(rhs_seg[:, :, 0], G2[:], qrb[:], OP.mult)
            # qmin' = q*min(s_self,1)/(b+1)
            V.tensor_tensor(rhs_seg[:, :, 1], s2[:], qrb[:], OP.mult)
            V.tensor_tensor(rhs_seg[:, :, 2], sb["valid"][:], rb1[:], OP.mult)
            V.tensor_tensor(rhs_seg[:, :, 3], pw[:], rb1[:], OP.mult)
            V.tensor_tensor(rhs_seg[:, :, 4], paypw[:], rb1[:], OP.mult)
            V.tensor_copy(rhs_seg[:, :, 5], qrb[:])

            # ---------- P5 loop 2: segment sums -> segPT rows [6, K] ---------
            segPT = accp.tile([6, K], f32, name="segPT")
            V.memset(segPT[:], 0.0)
            with tc.tile_pool(name="bmp6", bufs=3) as bmp6:
                for t in range(T):
                    if t < NPRE:
                        bmap = bm6pre[:, t, :]
                    else:
                        bm = bmp6.tile([P, K], f16, name="bm6")
                        V.tensor_scalar(bm[:], iotaH[:],
                                        sb["tidx"][:, t:t + 1],
                                        betap1[:, t:t + 1],
                                        OP.is_equal, OP.mult)
                        bmap = bm[:]
                    nc.tensor.matmul(segPT[:], rhs_seg[:, t, :], bmap,
                                     start=False, stop=(t == T - 1),
                                     skip_group_check=True)

            # ---------- P6: transpose seg rows to [P,KB,6] columns, then
            # AllReduce in p-outer layout (cheap 128-descriptor DMAs and a
            # column-parallel P7 instead of serial [1,K] row math) ----------
            segsbT = io.tile([6, K], f32, name="segsbT")
            SC.activation(segsbT[:], segPT[:], AF.Copy)
            repsb = io.tile([1, K], f32, name="repsb")
            SC.activation(repsb[:], repP[:], AF.Copy)
            seg_c = io.tile([P, KB, 6], f32, name="seg_c")
            for b in range(KB):
                tp = psA.tile([P, P], f32, name="tpseg", tag="tpose")
                nc.tensor.transpose(tp[:, 0:6],
                                    segsbT[0:6, b * P:(b + 1) * P],
                                    ident[0:6, 0:6])
                SC.activation(seg_c[:, b, :], tp[:, 0:6], AF.Copy)

            NSEG = 6 * K
            NTOT = NSEG + K + 4
            ar_in = dram.tile([1, NTOT], f32, name="ar_in")
            ar_out = dram.tile([1, NTOT], f32, name="ar_out", addr_space="Shared")
            nc.sync.dma_start(ar_in[0:1, 0:NSEG], seg_c[:])   # p-outer pack
            nc.sync.dma_start(ar_in[0:1, NSEG:NSEG + K], repsb[:])
            nc.sync.dma_start(ar_in[0:1, NSEG + K:NTOT], extrasF[:])
            if cc_mode == 'all':
                nc.gpsimd.collective_compute(
                    "AllReduce", OP.add,
                    replica_groups=[list(range(NCORES))],
                    ins=[ar_in[:]], outs=[ar_out[:]],
                )
            else:
                nc.sync.dma_start(ar_out[:], ar_in[:])
            seg_g = io.tile([P, KB, 6], f32, name="seg_g")
            nc.sync.dma_start(
                seg_g[:],
                ar_out[0:1, 0:NSEG].rearrange("o (p r) -> (o p) r", p=P))
            repB = io.tile([P, KB], f32, name="repB")
            for b in range(KB):
                nc.sync.dma_start(
                    repB[:, b:b + 1],
                    ar_out[0:1, NSEG + b * P:NSEG + (b + 1) * P])
            extras_g = io.tile([1, 4], f32, name="extras_g")
            nc.sync.dma_start(extras_g[:], ar_out[0:1, NSEG + K:NTOT])

            # ---------- P7: assembly ([P,KB] column math) ----------
            attseg = seg_g[:, :, 0]
            qminseg = seg_g[:, :, 1]
            count = seg_g[:, :, 2]
            pwseg = seg_g[:, :, 3]
            payseg = seg_g[:, :, 4]
            qseg = seg_g[:, :, 5]
            qaB = qab2[:, :, 0]
            BstarB = qab2[:, :, 1]

            def ntile(name):
                return io.tile([P, KB], f32, name=name)

            has = ntile("has")
            V.tensor_scalar(has[:], count, 0.0, None, OP.is_gt)
            rc = ntile("rc")        # 1/(count+eps)
            V.tensor_scalar(rc[:], count, EPS, None, OP.add)
            V.reciprocal(rc[:], rc[:])
            rnc = ntile("rnc")      # 1/(N-count+eps)
            V.tensor_scalar(rnc[:], count, -1.0, float(N) + EPS,
                            OP.mult, OP.add)
            V.reciprocal(rnc[:], rnc[:])

            la = ntile("la")        # qa*attseg/(count+eps) * has
            V.tensor_tensor(la[:], attseg, qaB, OP.mult)
            V.tensor_tensor(la[:], la[:], rc[:], OP.mult)
            V.tensor_tensor(la[:], la[:], has[:], OP.mult)

            # rep_k = (repB - qseg + qminseg)*qa*rnc*has   (relu-form)
            lr = ntile("lr")
            V.tensor_tensor(lr[:], repB[:], qminseg, OP.add)
            V.tensor_tensor(lr[:], lr[:], qseg, OP.subtract)
            V.tensor_tensor(lr[:], lr[:], qaB, OP.mult)
            V.tensor_tensor(lr[:], lr[:], rnc[:], OP.mult)
            V.tensor_tensor(lr[:], lr[:], has[:], OP.mult)

            lb = ntile("lb")        # has*(1 - beta_alpha) = has*(2 - Bstar)
            V.tensor_scalar(lb[:], BstarB, -1.0, 2.0, OP.mult, OP.add)
            V.tensor_tensor(lb[:], lb[:], has[:], OP.mult)

            lp = ntile("lp")        # has*paynum/(payden+eps)
            V.tensor_scalar(lp[:], pwseg, EPS, None, OP.add)
            V.reciprocal(lp[:], lp[:])
            V.tensor_tensor(lp[:], lp[:], payseg, OP.mult)
            V.tensor_tensor(lp[:], lp[:], has[:], OP.mult)

            asm = io.tile([P, 5], f32, name="asm")
            V.tensor_reduce(asm[:, 0:1], la[:], mybir.AxisListType.X, OP.add)
            V.tensor_reduce(asm[:, 1:2], lr[:], mybir.AxisListType.X, OP.add)
            V.tensor_reduce(asm[:, 2:3], lb[:], mybir.AxisListType.X, OP.add)
            V.tensor_reduce(asm[:, 3:4], lp[:], mybir.AxisListType.X, OP.add)
            V.tensor_reduce(asm[:, 4:5], has[:], mybir.AxisListType.X, OP.add)
            with tc.tile_pool(name="scpp", bufs=1, space="PSUM") as scp:
                sc2P = scp.tile([1, 5], f32, name="sc2P")
                nc.tensor.matmul(sc2P[:], onescol[:], asm[:],
                                 start=True, stop=True)
                fin = io.tile([1, 5], f32, name="fin")
                SC.activation(fin[:], sc2P[:], AF.Copy)

            # total = (la+lr+lb+lp)/n_obj + nb/(nn+eps) + 0.001*xsq/(2N)
            s4 = io.tile([1, 1], f32, name="s4")
            V.tensor_reduce(s4[:], fin[0:1, 0:4], mybir.AxisListType.X, OP.add)
            nobj = io.tile([1, 1], f32, name="nobj")
            V.tensor_scalar(nobj[:], fin[0:1, 4:5], EPS, None, OP.add)
            V.reciprocal(nobj[:], nobj[:])
            tot = io.tile([1, 1], f32, name="tot")
            V.tensor_tensor(tot[:], s4[:], nobj[:], OP.mult)
            nden = io.tile([1, 1], f32, name="nden")
            V.tensor_scalar(nden[:], extras_g[0:1, 1:2], EPS, None, OP.add)
            V.reciprocal(nden[:], nden[:])
            V.tensor_tensor(nden[:], nden[:], extras_g[0:1, 0:1], OP.mult)
            V.tensor_tensor(tot[:], tot[:], nden[:], OP.add)
            lcc = io.tile([1, 1], f32, name="lcc")
            SC.activation(lcc[:], extras_g[0:1, 2:3], AF.Copy,
                          scale=0.001 / (2.0 * N))
            V.tensor_tensor(tot[:], tot[:], lcc[:], OP.add)
            nc.sync.dma_start(out_d.ap(), tot[:])

    nc.compile()
    return nc


def _host_prep(inputs):
    """Slice, pad and re-layout the full inputs into 8 per-core input maps."""
    def lay(a2):                       # [SP, w] -> [128, T, w]
        w = a2.shape[1]
        r = a2.reshape(T, P, w).transpose(1, 0, 2)
        return np.ascontiguousarray(r.astype(np.float32))

    in_maps = []
    for c in range(NCORES):
        sl = slice(c * S, (c + 1) * S)

        def pad(a, fill=0.0):
            out = np.full((SP, a.shape[1]), fill, np.float32)
            out[:S] = a[sl]
            return out

        tidx = np.full((SP, 1), -2.0, np.float32)
        tidx[:S, 0] = inputs["t_idx"][sl, 0].astype(np.float32)
        valid = np.zeros((SP, 1), np.float32)
        valid[:S] = 1.0
        m = {
            "beta_r": lay(pad(inputs["pred_beta"]))[:, :, 0],
            "cc": lay(pad(inputs["pred_ccoords"])),
            "pE": lay(pad(inputs["pred_energy"]))[:, :, 0],
            "ppos": lay(pad(inputs["pred_pos"])),
            "ptime": lay(pad(inputs["pred_time"]))[:, :, 0],
            "pid": lay(pad(inputs["pred_id"])),
            "tE": lay(pad(inputs["t_energy"]))[:, :, 0],
            "tpos": lay(pad(inputs["t_pos"])),
            "ttime": lay(pad(inputs["t_time"]))[:, :, 0],
            "tidx": lay(tidx)[:, :, 0],
            "valid": lay(valid)[:, :, 0],
        }
        m = {k: np.ascontiguousarray(v) for k, v in m.items()}
        in_maps.append(m)
    return in_maps


def _run(inputs, trace=False):
    from concourse import bass_utils
    if "nc" not in _CACHE:
        _CACHE["nc"] = _build()
    nc = _CACHE["nc"]
    in_maps = _host_prep(inputs)
    res = bass_utils.run_bass_kernel_spmd(
        nc, in_maps, core_ids=list(range(NCORES)), trace=trace)
    return res


def kernel(**inputs):
    res = _run(inputs, trace=False)
    val = np.float32(res.results[0]["out"][0, 0])
    return np.array(val, dtype=np.float32)[()]


if __name__ == "__main__":
    d = np.load("/tmp/inputs.npz")
    inp = {k: d[k] for k in d.files}
    print("kernel:", kernel(**inp))


# revision 29
# speedup vs baseline: 1.0732x; 1.0732x over previous
"""Trainium2 Bass kernel for LLFullObjectCondensation loss (N=80000, K=512, C=2).

Strategy (8 NeuronCores, data-parallel over hits):
  - Each core gets a 10000-hit shard (padded to 79*128=10112), laid out [128, 79].
  - P1: per-hit quantities (q, payload, weights) as full-width [128,79] ops.
  - P2: per-object max of (beta+1)-weighted one-hot tiles (DVE build + running
        max), 4 PE transposes + reductions -> Bloc, packed to row BlocF [1,K].
  - AllReduce-max of BlocF; P3 selection pass overlaps: Isel = (bm == BlocB),
        PE matmul with sel3 stationary -> selPT [3,K] rows, gated by
        keepF = (BlocF == BglobF), AllReduce-add -> global x_alpha/q_alpha rows.
  - P5 loop 1 (software-pipelined): d2 via PE matmul (contract-4 trick), sqrt
        on ACT, min on DVE, repulsion row-sums via PE, self-distance gather via
        DVE row-reduce with the weighted one-hot.
  - P5 loop 2: segment sums as PE matmuls, rhs_seg [128,6] stationary, bm
        moving -> segPT [6,K] rows.
  - AllReduce-add of all per-object rows, then row-layout [1,K] assembly of
        the scalar loss.
All one-hot builds and elementwise work run on DVE/ACT (GPSIMD's software
tensor_scalar measured ~8us per [128,512] tile vs DVE's ~0.55us).
"""
import sys
import numpy as np

for _p in ("/opt/trn_rl_repo", "/root/.axon_site/_ro/trn_rl_repo"):
    if _p not in sys.path:
        sys.path.append(_p)

N = 80000
K = 512
NCORES = 8
S = N // NCORES          # 10000 hits per core
P = 128
T = 79                   # tiles per core, T*P = 10112 >= S
SP = T * P
KB = K // P              # 4 k-blocks
EPS = 1e-9
SQ_BIAS = 2e-2           # reference uses 1e-6; extra margin absorbs fp32r
                         # matmul rounding of the expanded |x|^2-2x.a+|a|^2
                         # form so sqrt never sees a negative input (measured:
                         # 1e-3 still went NaN -> fp32r product error ~5e-3 abs
                         # on O(20) terms; the 0.02 bias shifts the hinge by
                         # ~1.2e-3 relative on the total loss, gate is 2e-2)

_CACHE = {}


def _build(cc_mode='all'):
    import concourse.bass as bass
    import concourse.bacc as bacc
    import concourse.mybir as mybir
    import concourse.tile as tile
    from concourse import masks

    f32 = mybir.dt.float32
    f32r = mybir.dt.float32r
    i32 = mybir.dt.int32
    u16 = mybir.dt.uint16
    f16 = mybir.dt.float16
    AF = mybir.ActivationFunctionType
    OP = mybir.AluOpType

    nc = bacc.Bacc("TRN2", target_bir_lowering=False, debug=False,
                   num_devices=NCORES)

    di = {}
    def din(name, shape):
        di[name] = nc.dram_tensor(name, shape, f32, kind="ExternalInput")
        return di[name]

    din("beta_r", [P, T])
    din("cc", [P, T, 2])
    din("pE", [P, T])
    din("ppos", [P, T, 2])
    din("ptime", [P, T])
    din("pid", [P, T, 6])
    din("tE", [P, T])
    din("tpos", [P, T, 2])
    din("ttime", [P, T])
    din("tidx", [P, T])
    din("valid", [P, T])
    out_d = nc.dram_tensor("out", [1, 1], f32, kind="ExternalOutput")

    with tile.TileContext(nc) as tc:
        with (
            tc.tile_pool(name="const", bufs=1) as cpool,
            tc.tile_pool(name="io", bufs=1) as io,
            tc.tile_pool(name="dram", bufs=1, space="DRAM") as dram,
            tc.tile_pool(name="psA", bufs=2, space="PSUM") as psA,
            tc.tile_pool(name="acc", bufs=1, space="PSUM") as accp,
        ):
            # ---------- constants ----------
            ident = cpool.tile([P, P], f32)
            masks.make_identity(nc, ident[:])
            iotaI = cpool.tile([P, K], i32)
            nc.gpsimd.iota(iotaI[:], pattern=[[1, K]], base=0,
                           channel_multiplier=0)
            iotaF = cpool.tile([P, K], f32)
            nc.vector.tensor_copy(iotaF[:], iotaI[:])
            iotaH = cpool.tile([P, K], f16)
            nc.vector.tensor_copy(iotaH[:], iotaI[:])
            onescol = cpool.tile([P, 1], f32)
            nc.vector.memset(onescol[:], 1.0)
            onesrow = cpool.tile([1, P], f32)
            nc.vector.memset(onesrow[:], 1.0)

            _cb = {}
            def cbias(val):
                """[128,1] constant column for activation bias operands."""
                if val not in _cb:
                    ct = cpool.tile([P, 1], f32, name=f"cb{len(_cb)}")
                    nc.vector.memset(ct[:], val)
                    _cb[val] = ct
                return _cb[val][:]

            # ---------- load inputs ----------
            sb = {}
            for name, h in di.items():
                t_sb = io.tile(list(h.shape), f32, name=f"sb_{name}")
                nc.sync.dma_start(t_sb[:], h.ap())
                sb[name] = t_sb

            # ---------- P1: per-hit prep (all [128,T]-wide ops) ----------
            V = nc.vector
            SC = nc.scalar

            def wtile(name, shape=None, dtype=None):
                return io.tile(shape or [P, T], dtype or f32, name=name)
            u8 = mybir.dt.uint8

            beta = wtile("beta")
            V.tensor_scalar(beta[:], sb["beta_r"][:], 1e-6, 1.0 - 1e-6,
                            OP.max, OP.min)
            betap1 = wtile("betap1")
            SC.activation(betap1[:], beta[:], AF.Identity, bias=cbias(1.0))
            rb1 = wtile("rb1")
            V.reciprocal(rb1[:], betap1[:])
            onem = wtile("onem")
            SC.activation(onem[:], beta[:], AF.Identity, bias=cbias(1.0), scale=-1.0)
            recm = wtile("recm")
            V.reciprocal(recm[:], onem[:])
            ratio = wtile("ratio")
            V.tensor_tensor(ratio[:], betap1[:], recm[:], OP.mult)

            is_noise = wtile("is_noise")
            V.tensor_scalar(is_noise[:], sb["tidx"][:], -1.0, None, OP.is_equal)
            is_obj = wtile("is_obj")
            V.tensor_scalar(is_obj[:], sb["tidx"][:], 0.0, None, OP.is_ge)

            # energy weights w = relu(min(wr,1)) ; wr=(tE-0.5)/9.5
            wr = wtile("wr")
            SC.activation(wr[:], sb["tE"][:], AF.Identity, bias=cbias(-0.5 / 9.5),
                          scale=1.0 / 9.5)
            ew = wtile("ew")
            V.tensor_scalar(ew[:], wr[:], 1.0, 0.0, OP.min, OP.max)
            pw = wtile("pw")
            V.tensor_tensor(pw[:], beta[:], ew[:], OP.mult)
            V.tensor_tensor(pw[:], pw[:], is_obj[:], OP.mult)

            # --- energy loss pieces (pre-transcendental) ---
            ediff_r = wtile("ediff_r")
            V.tensor_tensor(ediff_r[:], sb["tE"][:], sb["pE"][:], OP.subtract)
            ediff = wtile("ediff")
            SC.activation(ediff[:], ediff_r[:], AF.Abs)
            ed2 = wtile("ed2")
            V.tensor_tensor(ed2[:], ediff[:], ediff[:], OP.mult)
            ed001 = wtile("ed001")
            SC.activation(ed001[:], ediff[:], AF.Copy, scale=0.001)

            # --- position loss pieces ---
            dpos = wtile("dpos", [P, T, 2])
            V.tensor_tensor(dpos[:], sb["tpos"][:], sb["ppos"][:], OP.subtract)
            V.tensor_tensor(dpos[:], dpos[:], dpos[:], OP.mult)
            d2p = wtile("d2p")
            V.tensor_tensor(d2p[:], dpos[:, :, 0], dpos[:, :, 1], OP.add)

            # --- timing loss pieces ---
            dtm = wtile("dtm")
            V.tensor_tensor(dtm[:], sb["ttime"][:], sb["ptime"][:], OP.subtract)
            adt = wtile("adt")
            SC.activation(adt[:], dtm[:], AF.Abs)
            dt2 = wtile("dt2")
            V.tensor_tensor(dt2[:], dtm[:], dtm[:], OP.mult)
            lint = wtile("lint")
            SC.activation(lint[:], adt[:], AF.Identity, bias=cbias(-4.0), scale=4.0)
            ltt = wtile("ltt", dtype=u8)
            V.tensor_scalar(ltt[:], adt[:], 2.0, None, OP.is_lt)
            ht = wtile("ht")
            V.select(ht[:], ltt[:], dt2[:], lint[:])
            yt = wtile("yt")
            SC.activation(yt[:], ht[:], AF.Copy, scale=1.0 / 6.0)

            # --- classification loss ---
            pid2 = wtile("pid2", [P, T, 6])
            V.tensor_tensor(pid2[:], sb["pid"][:], sb["pid"][:], OP.mult)
            cred = wtile("cred")
            V.tensor_reduce(cred[:], pid2[:], mybir.AxisListType.X, OP.add)

            # --- transcendental block: Exp, then Sqrt, then Ln (grouped to
            # limit ACT table swaps) ---
            ex = wtile("ex")
            SC.activation(ex[:], ed2[:], AF.Exp, scale=-0.1)
            xp = wtile("xp")
            SC.activation(xp[:], d2p[:], AF.Sqrt, bias=cbias(0.01), scale=0.01)

            lnr = wtile("lnr")
            SC.activation(lnr[:], ratio[:], AF.Ln)
            # q = (0.5*ln(ratio))^2 + 0.1, zeroed on padding
            halfln = wtile("halfln")
            SC.activation(halfln[:], lnr[:], AF.Copy, scale=0.5)
            q = wtile("q")
            V.tensor_tensor(q[:], halfln[:], halfln[:], OP.mult)
            V.scalar_tensor_tensor(q[:], q[:], 0.1, sb["valid"][:],
                                   OP.add, OP.mult)

            # energy softclip
            ye = wtile("ye")
            V.tensor_tensor(ye[:], ex[:], ed001[:], OP.add)
            lnye = wtile("lnye")
            SC.activation(lnye[:], ye[:], AF.Ln, bias=cbias(1.0))
            gte = wtile("gte", dtype=u8)
            V.tensor_scalar(gte[:], ye[:], 1.0, None, OP.is_gt)
            esc = wtile("esc")
            V.select(esc[:], gte[:], lnye[:], ye[:])

            # position huber + softclip
            xp2 = wtile("xp2")
            V.tensor_tensor(xp2[:], xp[:], xp[:], OP.mult)
            linp = wtile("linp")
            SC.activation(linp[:], xp[:], AF.Identity, bias=cbias(-100.0), scale=20.0)
            ltp = wtile("ltp", dtype=u8)
            V.tensor_scalar(ltp[:], xp[:], 10.0, None, OP.is_lt)
            hp = wtile("hp")
            V.select(hp[:], ltp[:], xp2[:], linp[:])
            yp = wtile("yp")
            SC.activation(yp[:], hp[:], AF.Copy, scale=1.0 / 3.0)
            lnyp = wtile("lnyp")
            SC.activation(lnyp[:], yp[:], AF.Ln, bias=cbias(1.0))
            gtp = wtile("gtp", dtype=u8)
            V.tensor_scalar(gtp[:], yp[:], 1.0, None, OP.is_gt)
            psc = wtile("psc")
            V.select(psc[:], gtp[:], lnyp[:], yp[:])

            # timing softclip
            lnyt = wtile("lnyt")
            SC.activation(lnyt[:], yt[:], AF.Ln, bias=cbias(1.0))
            gtt = wtile("gtt", dtype=u8)
            V.tensor_scalar(gtt[:], yt[:], 1.0, None, OP.is_gt)
            tsc = wtile("tsc")
            V.select(tsc[:], gtt[:], lnyt[:], yt[:])

            # payload = 10*esc + 3*psc + 6*tsc + (1e-8/6)*cred
            esc10 = wtile("esc10")
            SC.activation(esc10[:], esc[:], AF.Copy, scale=10.0)
            pay = wtile("pay")
            V.scalar_tensor_tensor(pay[:], psc[:], 3.0, esc10[:],
                                   OP.mult, OP.add)
            V.scalar_tensor_tensor(pay[:], tsc[:], 6.0, pay[:],
                                   OP.mult, OP.add)
            V.scalar_tensor_tensor(pay[:], cred[:], 1e-8 / 6.0, pay[:],
                                   OP.mult, OP.add)
            paypw = wtile("paypw")
            V.tensor_tensor(paypw[:], pay[:], pw[:], OP.mult)

            # selection rhs: [x0, x1, q] (stationary operand for P3 matmuls)
            sel3 = wtile("sel3", [P, T, 3], dtype=f32r)
            SC.activation(sel3[:, :, 0:2], sb["cc"][:], AF.Copy)
            V.tensor_copy(sel3[:, :, 2], q[:])

            # d2-matmul lhsT quantities [-2x0, -2x1, 1, |x|^2] packed [P,T,4]
            prep4 = wtile("prep4", [P, T, 4])
            SC.activation(prep4[:, :, 0:2], sb["cc"][:], AF.Copy, scale=-2.0)
            V.memset(prep4[:, :, 2], 1.0)
            ccsq = wtile("ccsq", [P, T, 2])
            V.tensor_tensor(ccsq[:], sb["cc"][:], sb["cc"][:], OP.mult)
            V.tensor_tensor(prep4[:, :, 3], ccsq[:, :, 0], ccsq[:, :, 1],
                            OP.add)

            # extras: [noise*beta, noise, |x|^2, q] free-reduced to [P,4],
            # then partition-reduced to a [1,4] row via PE (ready for AR3)
            extras = io.tile([P, 4], f32, name="extras")
            nb_t = wtile("nb_t")
            V.tensor_tensor(nb_t[:], is_noise[:], beta[:], OP.mult)
            V.tensor_reduce(extras[:, 0:1], nb_t[:], mybir.AxisListType.X, OP.add)
            V.tensor_reduce(extras[:, 1:2], is_noise[:], mybir.AxisListType.X, OP.add)
            V.tensor_reduce(extras[:, 2:3], prep4[:, :, 3], mybir.AxisListType.X, OP.add)
            V.tensor_reduce(extras[:, 3:4], q[:], mybir.AxisListType.X, OP.add)
            extrasF = io.tile([1, 4], f32, name="extrasF")
            with tc.tile_pool(name="exp", bufs=1, space="PSUM") as exp_p:
                exPS = exp_p.tile([1, 4], f32, name="exPS")
                nc.tensor.matmul(exPS[:], onescol[:], extras[:],
                                 start=True, stop=True)
                SC.activation(extrasF[:], exPS[:], AF.Copy)

            # transpose prep4 -> lhsT4r [4, T, 128] (f32r, rounded at the
            # ACT evacuation so the fp32r d2 matmul accepts it)
            lhsT4r = io.tile([4, T, P], f32r, name="lhsT4r")
            for r in range(4):
                tp = psA.tile([P, P], f32, name="tpose4", tag="tpose")
                nc.tensor.transpose(tp[0:T, :], prep4[:, :, r], ident[:])
                stage = io.tile([T, P], f32r, name=f"tstage{r}")
                SC.activation(stage[:], tp[0:T, :], AF.Copy)
                nc.sync.dma_start(lhsT4r[r:r + 1, :, :], stage[:])

            # ---------- P2: local per-object max of (beta+1)-weighted one-hot
            # (DVE builds; bm[p,k] = (iota[k]==tidx[p,t]) * (beta[p,t]+1)) ----
            runmax = io.tile([P, K], f32, name="runmax")
            V.memset(runmax[:], 0.0)
            with tc.tile_pool(name="bmp", bufs=3) as bmp:
                for t in range(T):
                    bm = bmp.tile([P, K], f32, name="bm")
                    V.tensor_scalar(bm[:], iotaF[:], sb["tidx"][:, t:t + 1],
                                    betap1[:, t:t + 1], OP.is_equal, OP.mult)
                    V.tensor_tensor(runmax[:], runmax[:], bm[:], OP.max)

            # partition-reduce runmax -> Bloc [128,4] (k = 128*b + p)
            Bloc = io.tile([P, KB], f32, name="Bloc")
            for b in range(KB):
                tp = psA.tile([P, P], f32, name="tpose", tag="tpose")
                nc.tensor.transpose(tp[:], runmax[:, b * P:(b + 1) * P], ident[:])
                V.reduce_max(Bloc[:, b:b + 1], tp[:], axis=mybir.AxisListType.X)

            # row layout: BlocF[0, 128*b+p] = Bloc[p, b]
            BlocF = io.tile([1, K], f32, name="BlocF")
            for b in range(KB):
                nc.sync.dma_start(BlocF[0:1, b * P:(b + 1) * P], Bloc[:, b:b + 1])

            # ---------- P4a: AllReduce-max of BlocF (overlaps with P3) -------
            arm_in = dram.tile([1, K], f32, name="arm_in")
            arm_out = dram.tile([1, K], f32, name="arm_out", addr_space="Shared")
            nc.sync.dma_start(arm_in[0:1, :], BlocF[:])
            if cc_mode in ('all', 'first', 'two'):
                nc.gpsimd.collective_compute(
                    "AllReduce", OP.max,
                    replica_groups=[list(range(NCORES))],
                    ins=[arm_in[:]], outs=[arm_out[:]],
                )
            else:
                nc.sync.dma_start(arm_out[:], arm_in[:])
            BglobF = io.tile([1, K], f32, name="BglobF")
            nc.sync.dma_start(BglobF[:], arm_out[0:1, :])

            # broadcast BlocF across partitions via PE: ones[1,P].T @ BlocF
            BlocB = io.tile([P, K], f32, name="BlocB")
            with tc.tile_pool(name="bcp", bufs=1, space="PSUM") as bcp:
                blocps = bcp.tile([P, K], f32, name="blocps")
                nc.tensor.matmul(blocps[:], onesrow[:], BlocF[:],
                                 start=True, stop=True)
                SC.activation(BlocB[:], blocps[:], AF.Copy)

            # ---------- P3: selection segment-sums -> selPT rows [3, K] ------
            with (
                tc.tile_pool(name="selpp", bufs=1, space="PSUM") as selpp,
                tc.tile_pool(name="bmp3", bufs=3) as bmp3,
            ):
                selPT = selpp.tile([3, K], f32, name="selPT")
                V.memset(selPT[:], 0.0)
                for t in range(T):
                    bm = bmp3.tile([P, K], f32, name="bm3")
                    V.tensor_scalar(bm[:], iotaF[:], sb["tidx"][:, t:t + 1],
                                    betap1[:, t:t + 1], OP.is_equal, OP.mult)
                    Isel = bmp3.tile([P, K], f32r, name="Isel")
                    V.tensor_tensor(Isel[:], bm[:], BlocB[:], OP.is_equal)
                    nc.tensor.matmul(selPT[:], sel3[:, t, :], Isel[:],
                                     start=False, stop=(t == T - 1),
                                     skip_group_check=True)
                selsbT = io.tile([3, K], f32, name="selsbT")
                SC.activation(selsbT[:], selPT[:], AF.Copy)

            # gate by global-winner mask and AllReduce-add.  Compute engines
            # must start at partition 0/32/64/96, so replicate keepF to 3
            # partitions via DMA and gate with one [3,K] multiply.
            keepF = io.tile([1, K], f32, name="keepF")
            V.tensor_tensor(keepF[:], BlocF[:], BglobF[:], OP.is_equal)
            keep3 = io.tile([3, K], f32, name="keep3")
            for r in range(3):
                nc.sync.dma_start(keep3[r:r + 1, :], keepF[:])
            sel_cT = io.tile([3, K], f32, name="sel_cT")
            V.tensor_tensor(sel_cT[:], selsbT[:], keep3[:], OP.mult)
            ar2_in = dram.tile([1, 3 * K], f32, name="ar2_in")
            ar2_out = dram.tile([1, 3 * K], f32, name="ar2_out",
                                addr_space="Shared")
            nc.sync.dma_start(ar2_in[0:1, :], sel_cT[:])   # row-major pack
            if cc_mode in ('all', 'two'):
                nc.gpsimd.collective_compute(
                    "AllReduce", OP.add,
                    replica_groups=[list(range(NCORES))],
                    ins=[ar2_in[:]], outs=[ar2_out[:]],
                )
            else:
                nc.sync.dma_start(ar2_out[:], ar2_in[:])

            # rhsD2 rows: [xa0; xa1; |xa|^2; 1].  |xa|^2 is computed on
            # partition 0 (xa0F/xa1F row tiles) and DMA'd into row 2.
            rhsD2 = io.tile([4, K], f32, name="rhsD2")
            V.memset(rhsD2[:], 1.0)
            nc.sync.dma_start(
                rhsD2[0:2, :],
                ar2_out[0:1, 0:2 * K].rearrange("o (r k) -> (o r) k", r=2))
            xa0F = io.tile([1, K], f32, name="xa0F")
            nc.sync.dma_start(xa0F[:], ar2_out[0:1, 0:K])
            xa1F = io.tile([1, K], f32, name="xa1F")
            nc.sync.dma_start(xa1F[:], ar2_out[0:1, K:2 * K])
            qaF = io.tile([1, K], f32, name="qaF")
            nc.sync.dma_start(qaF[:], ar2_out[0:1, 2 * K:3 * K])
            xsqF = io.tile([1, K], f32, name="xsqF")
            xsq_t = io.tile([1, K], f32, name="xsq_t")
            V.tensor_tensor(xsq_t[:], xa1F[:], xa1F[:], OP.mult)
            V.tensor_tensor(xsqF[:], xa0F[:], xa0F[:], OP.mult)
            V.tensor_tensor(xsqF[:], xsqF[:], xsq_t[:], OP.add)
            nc.sync.dma_start(rhsD2[2:3, :], xsqF[:])
            rhsD2r = io.tile([4, K], f32r, name="rhsD2r")
            V.tensor_copy(rhsD2r[:], rhsD2[:])

            # prebuild the first NPRE segment one-hots around the AR2 window
            NPRE = 32
            bm6pre = io.tile([P, NPRE, K], f16, name="bm6pre")
            for t in range(NPRE):
                V.tensor_scalar(bm6pre[:, t, :], iotaH[:],
                                sb["tidx"][:, t:t + 1],
                                betap1[:, t:t + 1], OP.is_equal, OP.mult)

            qr = wtile("qr", dtype=f32r)      # rounded copy for fp32r matmul
            V.tensor_copy(qr[:], q[:])

            # ---------- P5 loop 1: d2 block, rep row-sums, self-distance -----
            # software-pipelined by one stage: d2 matmul for t+1 issues before
            # the rep matmul for t so the PE never blocks behind the sqrt.
            # rep accumulates q * relu(1 - s) directly (hinge on ACT).
            gstD = io.tile([P, T], f32, name="gstD")   # (beta+1)*s_self
            repP = accp.tile([1, K], f32, name="repP")
            V.memset(repP[:], 0.0)
            scr = io.tile([P, K], f16, name="scr")         # ttr full-out scratch
            with (
                tc.tile_pool(name="d2pool", bufs=3, space="PSUM") as d2pool,
                tc.tile_pool(name="sp", bufs=3) as sp,
                tc.tile_pool(name="bmp5", bufs=3) as bmp5,
            ):
                d2tiles = {}
                def d2mm(t):
                    d2ps = d2pool.tile([P, K], f32, name="d2ps")
                    nc.tensor.matmul(d2ps[:], lhsT4r[0:4, t, :], rhsD2r[:],
                                     start=True, stop=True)
                    d2tiles[t] = d2ps
                d2mm(0)
                for t in range(T):
                    if t + 1 < T:
                        d2mm(t + 1)
                    d2ps = d2tiles.pop(t)
                    bm = bmp5.tile([P, K], f16, name="bm5")
                    V.tensor_scalar(bm[:], iotaH[:], sb["tidx"][:, t:t + 1],
                                    betap1[:, t:t + 1], OP.is_equal, OP.mult)
                    sS = sp.tile([P, K], f16, name="sS")
                    SC.activation(sS[:], d2ps[:], AF.Sqrt, bias=cbias(SQ_BIAS))
                    rlu = sp.tile([P, K], f32r, name="rlu")
                    SC.activation(rlu[:], sS[:], AF.Relu, bias=cbias(1.0),
                                  scale=-1.0)
                    nc.tensor.matmul(repP[:], qr[:, t:t + 1], rlu[:],
                                     start=False, stop=(t == T - 1),
                                     skip_group_check=True)
                    V.scalar_tensor_tensor(
                        scr[:], bm[:], 1.0, sS[:], OP.bypass, OP.mult,
                        accum_out=gstD[:, t:t + 1])

            # ---------- global per-hit math for segment rhs ----------
            qrb = wtile("qrb")
            V.tensor_tensor(qrb[:], q[:], rb1[:], OP.mult)
            sself = wtile("sself")              # sqrt(d2_self + SQ_BIAS)
            V.tensor_tensor(sself[:], gstD[:], rb1[:], OP.mult)
            G2 = wtile("G2")                    # d2_self
            V.tensor_tensor(G2[:], sself[:], sself[:], OP.mult)
            V.tensor_scalar(G2[:], G2[:], SQ_BIAS, None, OP.subtract)
            s2 = wtile("s2")                    # min(s_self, 1)
            V.tensor_scalar(s2[:], sself[:], 1.0, None, OP.min)
            rhs_seg = io.tile([P, T, 6], f16, name="rhs_seg")
            # att' = q*d2_self/(b+1)
            V.tensor_tensor(rhs_seg[:, :, 0], G2[:], qrb[:], OP.mult)
            # qmin' = q*min(s_self,1)/(b+1)
            V.tensor_tensor(rhs_seg[:, :, 1], s2[:], qrb[:], OP.mult)
            V.tensor_tensor(rhs_seg[:, :, 2], sb["valid"][:], rb1[:], OP.mult)
            V.tensor_tensor(rhs_seg[:, :, 3], pw[:], rb1[:], OP.mult)
            V.tensor_tensor(rhs_seg[:, :, 4], paypw[:], rb1[:], OP.mult)
            V.tensor_copy(rhs_seg[:, :, 5], qrb[:])

            # ---------- P5 loop 2: segment sums -> segPT rows [6, K] ---------
            segPT = accp.tile([6, K], f32, name="segPT")
            V.memset(segPT[:], 0.0)
            with tc.tile_pool(name="bmp6", bufs=3) as bmp6:
                for t in range(T):
                    if t < NPRE:
                        bmap = bm6pre[:, t, :]
                    else:
                        bm = bmp6.tile([P, K], f16, name="bm6")
                        V.tensor_scalar(bm[:], iotaH[:],
                                        sb["tidx"][:, t:t + 1],
                                        betap1[:, t:t + 1],
                                        OP.is_equal, OP.mult)
                        bmap = bm[:]
                    nc.tensor.matmul(segPT[:], rhs_seg[:, t, :], bmap,
                                     start=False, stop=(t == T - 1),
                                     skip_group_check=True)

            # ---------- P6: AllReduce of per-object rows ----------
            segsbT = io.tile([6, K], f32, name="segsbT")
            SC.activation(segsbT[:], segPT[:], AF.Copy)
            repsb = io.tile([1, K], f32, name="repsb")
            SC.activation(repsb[:], repP[:], AF.Copy)

            NSEG = 6 * K
            NTOT = NSEG + K + 4
            ar_in = dram.tile([1, NTOT], f32, name="ar_in")
            ar_out = dram.tile([1, NTOT], f32, name="ar_out", addr_space="Shared")
            nc.sync.dma_start(ar_in[0:1, 0:NSEG], segsbT[:])     # row-major
            nc.sync.dma_start(ar_in[0:1, NSEG:NSEG + K], repsb[:])
            nc.sync.dma_start(ar_in[0:1, NSEG + K:NTOT], extrasF[:])
            if cc_mode == 'all':
                nc.gpsimd.collective_compute(
                    "AllReduce", OP.add,
                    replica_groups=[list(range(NCORES))],
                    ins=[ar_in[:]], outs=[ar_out[:]],
                )
            else:
                nc.sync.dma_start(ar_out[:], ar_in[:])
            # unpack per-object rows (partition-0 tiles; compute engines
            # cannot start at partitions 1..5)
            seg_rows = []
            for r in range(6):
                rt = io.tile([1, K], f32, name=f"segrow{r}")
                nc.sync.dma_start(rt[:], ar_out[0:1, r * K:(r + 1) * K])
                seg_rows.append(rt)
            repF = io.tile([1, K], f32, name="repF")
            nc.sync.dma_start(repF[:], ar_out[0:1, NSEG:NSEG + K])
            extras_g = io.tile([1, 4], f32, name="extras_g")
            nc.sync.dma_start(extras_g[:], ar_out[0:1, NSEG + K:NTOT])

            # pre-reciprocal rows packed onto partitions 0-11 as [12,128]
            # (one 128-wide reciprocal instead of three 512-wide ones)
            rci = io.tile([12, P], f32, name="rci")
            rco = io.tile([12, P], f32, name="rco")

            # ---------- P7: assembly ([1,K] rows) ----------
            attseg = seg_rows[0][:]
            qminseg = seg_rows[1][:]
            count = seg_rows[2][:]
            pwseg = seg_rows[3][:]
            payseg = seg_rows[4][:]
            qseg = seg_rows[5][:]

            def rtile(name):
                return io.tile([1, K], f32, name=name)

            has = rtile("has")
            V.tensor_scalar(has[:], count, 0.0, None, OP.is_gt)
            # denominators: count+eps, N-count+eps, pwseg+eps -> one packed
            # reciprocal on partitions 0-11
            den0 = rtile("den0")
            V.tensor_scalar(den0[:], count, EPS, None, OP.add)
            den1 = rtile("den1")
            V.tensor_scalar(den1[:], count, -1.0, float(N) + EPS,
                            OP.mult, OP.add)
            den2 = rtile("den2")
            V.tensor_scalar(den2[:], pwseg, EPS, None, OP.add)
            nc.sync.dma_start(rci[0:4, :], den0[:])
            nc.sync.dma_start(rci[4:8, :], den1[:])
            nc.sync.dma_start(rci[8:12, :], den2[:])
            V.reciprocal(rco[:], rci[:])
            rc = rtile("rc")
            nc.sync.dma_start(rc[:], rco[0:4, :])
            rnc = rtile("rnc")
            nc.sync.dma_start(rnc[:], rco[4:8, :])
            lpd = rtile("lpd")
            nc.sync.dma_start(lpd[:], rco[8:12, :])

            la = rtile("la")        # qa*attseg/(count+eps) * has
            V.tensor_tensor(la[:], attseg, qaF[:], OP.mult)
            V.tensor_tensor(la[:], la[:], rc[:], OP.mult)
            V.tensor_tensor(la[:], la[:], has[:], OP.mult)

            # rep_k = (repF - qseg + qminseg)*qa*rnc*has   (relu-form)
            lr = rtile("lr")
            V.tensor_tensor(lr[:], repF[:], qminseg, OP.add)
            V.tensor_tensor(lr[:], lr[:], qseg, OP.subtract)
            V.tensor_tensor(lr[:], lr[:], qaF[:], OP.mult)
            V.tensor_tensor(lr[:], lr[:], rnc[:], OP.mult)
            V.tensor_tensor(lr[:], lr[:], has[:], OP.mult)

            lb = rtile("lb")        # has*(1 - beta_alpha) = has*(2 - Bglob)
            V.tensor_scalar(lb[:], BglobF[:], -1.0, 2.0, OP.mult, OP.add)
            V.tensor_tensor(lb[:], lb[:], has[:], OP.mult)

            lp = rtile("lp")        # has*paynum/(payden+eps)
            V.tensor_tensor(lp[:], lpd[:], payseg, OP.mult)
            V.tensor_tensor(lp[:], lp[:], has[:], OP.mult)

            lsum = rtile("lsum")
            V.tensor_tensor(lsum[:], la[:], lr[:], OP.add)
            V.tensor_tensor(lsum[:], lsum[:], lb[:], OP.add)
            V.tensor_tensor(lsum[:], lsum[:], lp[:], OP.add)
            fin = io.tile([1, 2], f32, name="fin")
            V.tensor_reduce(fin[0:1, 0:1], lsum[:], mybir.AxisListType.X, OP.add)
            V.tensor_reduce(fin[0:1, 1:2], has[:], mybir.AxisListType.X, OP.add)

            # total = lsum/n_obj + nb/(nn+eps) + 0.001*xsq/(2N)
            nobj = io.tile([1, 1], f32, name="nobj")
            V.tensor_scalar(nobj[:], fin[0:1, 1:2], EPS, None, OP.add)
            V.reciprocal(nobj[:], nobj[:])
            tot = io.tile([1, 1], f32, name="tot")
            V.tensor_tensor(tot[:], fin[0:1, 0:1], nobj[:], OP.mult)
            nden = io.tile([1, 1], f32, name="nden")
            V.tensor_scalar(nden[:], extras_g[0:1, 1:2], EPS, None, OP.add)
            V.reciprocal(nden[:], nden[:])
            V.tensor_tensor(nden[:], nden[:], extras_g[0:1, 0:1], OP.mult)
            V.tensor_tensor(tot[:], tot[:], nden[:], OP.add)
            lcc = io.tile([1, 1], f32, name="lcc")
            SC.activation(lcc[:], extras_g[0:1, 2:3], AF.Copy,
                          scale=0.001 / (2.0 * N))
            V.tensor_tensor(tot[:], tot[:], lcc[:], OP.add)
            nc.sync.dma_start(out_d.ap(), tot[:])

    nc.compile()
    return nc


def _host_prep(inputs):
    """Slice, pad and re-layout the full inputs into 8 per-core input maps."""
    def lay(a2):                       # [SP, w] -> [128, T, w]
        w = a2.shape[1]
        r = a2.reshape(T, P, w).transpose(1, 0, 2)
        return np.ascontiguousarray(r.astype(np.float32))

    in_maps = []
    for c in range(NCORES):
        sl = slice(c * S, (c + 1) * S)

        def pad(a, fill=0.0):
            out = np.full((SP, a.shape[1]), fill, np.float32)
            out[:S] = a[sl]
            return out

        tidx = np.full((SP, 1), -2.0, np.float32)
        tidx[:S, 0] = inputs["t_idx"][sl, 0].astype(np.float32)
        valid = np.zeros((SP, 1), np.float32)
        valid[:S] = 1.0
        m = {
            "beta_r": lay(pad(inputs["pred_beta"]))[:, :, 0],
            "cc": lay(pad(inputs["pred_ccoords"])),
            "pE": lay(pad(inputs["pred_energy"]))[:, :, 0],
            "ppos": lay(pad(inputs["pred_pos"])),
            "ptime": lay(pad(inputs["pred_time"]))[:, :, 0],
            "pid": lay(pad(inputs["pred_id"])),
            "tE": lay(pad(inputs["t_energy"]))[:, :, 0],
            "tpos": lay(pad(inputs["t_pos"])),
            "ttime": lay(pad(inputs["t_time"]))[:, :, 0],
            "tidx": lay(tidx)[:, :, 0],
            "valid": lay(valid)[:, :, 0],
        }
        m = {k: np.ascontiguousarray(v) for k, v in m.items()}
        in_maps.append(m)
    return in_maps


def _run(inputs, trace=False):
    from concourse import bass_utils
    if "nc" not in _CACHE:
        _CACHE["nc"] = _build()
    nc = _CACHE["nc"]
    in_maps = _host_prep(inputs)
    res = bass_utils.run_bass_kernel_spmd(
        nc, in_maps, core_ids=list(range(NCORES)), trace=trace)
    return res


def kernel(**inputs):
    res = _run(inputs, trace=False)
    val = np.float32(res.results[0]["out"][0, 0])
    return np.array(val, dtype=np.float32)[()]


if __name__ == "__main__":
    d = np.load("/tmp/inputs.npz")
    inp = {k: d[k] for k in d.files}
    print("kernel:", kernel(**inp))


# revision 31
# speedup vs baseline: 1.0998x; 1.0248x over previous
"""Trainium2 Bass kernel for LLFullObjectCondensation loss (N=80000, K=512, C=2).

Strategy (8 NeuronCores, data-parallel over hits):
  - Each core gets a 10000-hit shard (padded to 79*128=10112), laid out [128, 79].
  - P1: per-hit quantities (q, payload, weights) as full-width [128,79] ops.
  - P2: per-object max of (beta+1)-weighted one-hot tiles (DVE build + running
        max), 4 PE transposes + reductions -> Bloc, packed to row BlocF [1,K].
  - AllReduce-max of BlocF; P3 selection pass overlaps: Isel = (bm == BlocB),
        PE matmul with sel3 stationary -> selPT [3,K] rows, gated by
        keepF = (BlocF == BglobF), AllReduce-add -> global x_alpha/q_alpha rows.
  - P5 loop 1 (software-pipelined): d2 via PE matmul (contract-4 trick), sqrt
        on ACT, min on DVE, repulsion row-sums via PE, self-distance gather via
        DVE row-reduce with the weighted one-hot.
  - P5 loop 2: segment sums as PE matmuls, rhs_seg [128,6] stationary, bm
        moving -> segPT [6,K] rows.
  - AllReduce-add of all per-object rows, then row-layout [1,K] assembly of
        the scalar loss.
All one-hot builds and elementwise work run on DVE/ACT (GPSIMD's software
tensor_scalar measured ~8us per [128,512] tile vs DVE's ~0.55us).
"""
import sys
import numpy as np

for _p in ("/opt/trn_rl_repo", "/root/.axon_site/_ro/trn_rl_repo"):
    if _p not in sys.path:
        sys.path.append(_p)

N = 80000
K = 512
NCORES = 8
S = N // NCORES          # 10000 hits per core
P = 128
T = 79                   # tiles per core, T*P = 10112 >= S
SP = T * P
KB = K // P              # 4 k-blocks
EPS = 1e-9
SQ_BIAS = 2e-2           # reference uses 1e-6; extra margin absorbs fp32r
                         # matmul rounding of the expanded |x|^2-2x.a+|a|^2
                         # form so sqrt never sees a negative input (measured:
                         # 1e-3 still went NaN -> fp32r product error ~5e-3 abs
                         # on O(20) terms; the 0.02 bias shifts the hinge by
                         # ~1.2e-3 relative on the total loss, gate is 2e-2)

_CACHE = {}


def _build(cc_mode='all'):
    import concourse.bass as bass
    import concourse.bacc as bacc
    import concourse.mybir as mybir
    import concourse.tile as tile
    from concourse import masks

    f32 = mybir.dt.float32
    f32r = mybir.dt.float32r
    i32 = mybir.dt.int32
    u16 = mybir.dt.uint16
    f16 = mybir.dt.float16
    AF = mybir.ActivationFunctionType
    OP = mybir.AluOpType

    nc = bacc.Bacc("TRN2", target_bir_lowering=False, debug=False,
                   num_devices=NCORES)

    di = {}
    def din(name, shape):
        di[name] = nc.dram_tensor(name, shape, f32, kind="ExternalInput")
        return di[name]

    din("beta_r", [P, T])
    din("cc", [P, T, 2])
    din("pE", [P, T])
    din("ppos", [P, T, 2])
    din("ptime", [P, T])
    din("pid", [P, T, 6])
    din("tE", [P, T])
    din("tpos", [P, T, 2])
    din("ttime", [P, T])
    din("tidx", [P, T])
    din("valid", [P, T])
    out_d = nc.dram_tensor("out", [1, 1], f32, kind="ExternalOutput")

    with tile.TileContext(nc) as tc:
        with (
            tc.tile_pool(name="const", bufs=1) as cpool,
            tc.tile_pool(name="io", bufs=1) as io,
            tc.tile_pool(name="dram", bufs=1, space="DRAM") as dram,
            tc.tile_pool(name="psA", bufs=2, space="PSUM") as psA,
            tc.tile_pool(name="acc", bufs=1, space="PSUM") as accp,
        ):
            # ---------- constants ----------
            ident = cpool.tile([P, P], f32)
            masks.make_identity(nc, ident[:])
            iotaI = cpool.tile([P, K], i32)
            nc.gpsimd.iota(iotaI[:], pattern=[[1, K]], base=0,
                           channel_multiplier=0)
            iotaF = cpool.tile([P, K], f32)
            nc.vector.tensor_copy(iotaF[:], iotaI[:])
            iotaH = cpool.tile([P, K], f16)
            nc.vector.tensor_copy(iotaH[:], iotaI[:])
            onescol = cpool.tile([P, 1], f32)
            nc.vector.memset(onescol[:], 1.0)
            onesrow = cpool.tile([1, P], f32)
            nc.vector.memset(onesrow[:], 1.0)

            _cb = {}
            def cbias(val):
                """[128,1] constant column for activation bias operands."""
                if val not in _cb:
                    ct = cpool.tile([P, 1], f32, name=f"cb{len(_cb)}")
                    nc.vector.memset(ct[:], val)
                    _cb[val] = ct
                return _cb[val][:]

            # ---------- load inputs ----------
            sb = {}
            for name, h in di.items():
                t_sb = io.tile(list(h.shape), f32, name=f"sb_{name}")
                nc.sync.dma_start(t_sb[:], h.ap())
                sb[name] = t_sb

            # ---------- P1: per-hit prep (all [128,T]-wide ops) ----------
            V = nc.vector
            SC = nc.scalar

            def wtile(name, shape=None, dtype=None):
                return io.tile(shape or [P, T], dtype or f32, name=name)
            u8 = mybir.dt.uint8

            beta = wtile("beta")
            V.tensor_scalar(beta[:], sb["beta_r"][:], 1e-6, 1.0 - 1e-6,
                            OP.max, OP.min)
            betap1 = wtile("betap1")
            SC.activation(betap1[:], beta[:], AF.Identity, bias=cbias(1.0))
            rb1 = wtile("rb1")
            V.reciprocal(rb1[:], betap1[:])
            onem = wtile("onem")
            SC.activation(onem[:], beta[:], AF.Identity, bias=cbias(1.0), scale=-1.0)
            recm = wtile("recm")
            V.reciprocal(recm[:], onem[:])
            ratio = wtile("ratio")
            V.tensor_tensor(ratio[:], betap1[:], recm[:], OP.mult)

            is_noise = wtile("is_noise")
            V.tensor_scalar(is_noise[:], sb["tidx"][:], -1.0, None, OP.is_equal)
            is_obj = wtile("is_obj")
            V.tensor_scalar(is_obj[:], sb["tidx"][:], 0.0, None, OP.is_ge)

            # energy weights w = relu(min(wr,1)) ; wr=(tE-0.5)/9.5
            wr = wtile("wr")
            SC.activation(wr[:], sb["tE"][:], AF.Identity, bias=cbias(-0.5 / 9.5),
                          scale=1.0 / 9.5)
            ew = wtile("ew")
            V.tensor_scalar(ew[:], wr[:], 1.0, 0.0, OP.min, OP.max)
            pw = wtile("pw")
            V.tensor_tensor(pw[:], beta[:], ew[:], OP.mult)
            V.tensor_tensor(pw[:], pw[:], is_obj[:], OP.mult)

            # --- energy loss pieces (pre-transcendental) ---
            ediff_r = wtile("ediff_r")
            V.tensor_tensor(ediff_r[:], sb["tE"][:], sb["pE"][:], OP.subtract)
            ediff = wtile("ediff")
            SC.activation(ediff[:], ediff_r[:], AF.Abs)
            ed2 = wtile("ed2")
            V.tensor_tensor(ed2[:], ediff[:], ediff[:], OP.mult)
            ed001 = wtile("ed001")
            SC.activation(ed001[:], ediff[:], AF.Copy, scale=0.001)

            # --- position loss pieces ---
            dpos = wtile("dpos", [P, T, 2])
            V.tensor_tensor(dpos[:], sb["tpos"][:], sb["ppos"][:], OP.subtract)
            V.tensor_tensor(dpos[:], dpos[:], dpos[:], OP.mult)
            d2p = wtile("d2p")
            V.tensor_tensor(d2p[:], dpos[:, :, 0], dpos[:, :, 1], OP.add)

            # --- timing loss pieces ---
            dtm = wtile("dtm")
            V.tensor_tensor(dtm[:], sb["ttime"][:], sb["ptime"][:], OP.subtract)
            adt = wtile("adt")
            SC.activation(adt[:], dtm[:], AF.Abs)
            dt2 = wtile("dt2")
            V.tensor_tensor(dt2[:], dtm[:], dtm[:], OP.mult)
            lint = wtile("lint")
            SC.activation(lint[:], adt[:], AF.Identity, bias=cbias(-4.0), scale=4.0)
            ltt = wtile("ltt", dtype=u8)
            V.tensor_scalar(ltt[:], adt[:], 2.0, None, OP.is_lt)
            ht = wtile("ht")
            V.select(ht[:], ltt[:], dt2[:], lint[:])
            yt = wtile("yt")
            SC.activation(yt[:], ht[:], AF.Copy, scale=1.0 / 6.0)

            # --- classification loss ---
            pid2 = wtile("pid2", [P, T, 6])
            V.tensor_tensor(pid2[:], sb["pid"][:], sb["pid"][:], OP.mult)
            cred = wtile("cred")
            V.tensor_reduce(cred[:], pid2[:], mybir.AxisListType.X, OP.add)

            # --- transcendental block: Exp, then Sqrt, then Ln (grouped to
            # limit ACT table swaps) ---
            ex = wtile("ex")
            SC.activation(ex[:], ed2[:], AF.Exp, scale=-0.1)
            xp = wtile("xp")
            SC.activation(xp[:], d2p[:], AF.Sqrt, bias=cbias(0.01), scale=0.01)

            lnr = wtile("lnr")
            SC.activation(lnr[:], ratio[:], AF.Ln)
            # q = (0.5*ln(ratio))^2 + 0.1, zeroed on padding
            halfln = wtile("halfln")
            SC.activation(halfln[:], lnr[:], AF.Copy, scale=0.5)
            q = wtile("q")
            V.tensor_tensor(q[:], halfln[:], halfln[:], OP.mult)
            V.scalar_tensor_tensor(q[:], q[:], 0.1, sb["valid"][:],
                                   OP.add, OP.mult)

            # energy softclip
            ye = wtile("ye")
            V.tensor_tensor(ye[:], ex[:], ed001[:], OP.add)
            lnye = wtile("lnye")
            SC.activation(lnye[:], ye[:], AF.Ln, bias=cbias(1.0))
            gte = wtile("gte", dtype=u8)
            V.tensor_scalar(gte[:], ye[:], 1.0, None, OP.is_gt)
            esc = wtile("esc")
            V.select(esc[:], gte[:], lnye[:], ye[:])

            # position huber + softclip
            xp2 = wtile("xp2")
            V.tensor_tensor(xp2[:], xp[:], xp[:], OP.mult)
            linp = wtile("linp")
            SC.activation(linp[:], xp[:], AF.Identity, bias=cbias(-100.0), scale=20.0)
            ltp = wtile("ltp", dtype=u8)
            V.tensor_scalar(ltp[:], xp[:], 10.0, None, OP.is_lt)
            hp = wtile("hp")
            V.select(hp[:], ltp[:], xp2[:], linp[:])
            yp = wtile("yp")
            SC.activation(yp[:], hp[:], AF.Copy, scale=1.0 / 3.0)
            lnyp = wtile("lnyp")
            SC.activation(lnyp[:], yp[:], AF.Ln, bias=cbias(1.0))
            gtp = wtile("gtp", dtype=u8)
            V.tensor_scalar(gtp[:], yp[:], 1.0, None, OP.is_gt)
            psc = wtile("psc")
            V.select(psc[:], gtp[:], lnyp[:], yp[:])

            # timing softclip
            lnyt = wtile("lnyt")
            SC.activation(lnyt[:], yt[:], AF.Ln, bias=cbias(1.0))
            gtt = wtile("gtt", dtype=u8)
            V.tensor_scalar(gtt[:], yt[:], 1.0, None, OP.is_gt)
            tsc = wtile("tsc")
            V.select(tsc[:], gtt[:], lnyt[:], yt[:])

            # payload = 10*esc + 3*psc + 6*tsc + (1e-8/6)*cred
            esc10 = wtile("esc10")
            SC.activation(esc10[:], esc[:], AF.Copy, scale=10.0)
            pay = wtile("pay")
            V.scalar_tensor_tensor(pay[:], psc[:], 3.0, esc10[:],
                                   OP.mult, OP.add)
            V.scalar_tensor_tensor(pay[:], tsc[:], 6.0, pay[:],
                                   OP.mult, OP.add)
            V.scalar_tensor_tensor(pay[:], cred[:], 1e-8 / 6.0, pay[:],
                                   OP.mult, OP.add)
            paypw = wtile("paypw")
            V.tensor_tensor(paypw[:], pay[:], pw[:], OP.mult)

            # selection rhs: [x0, x1, q] (stationary operand for P3 matmuls)
            sel3 = wtile("sel3", [P, T, 3], dtype=f32r)
            SC.activation(sel3[:, :, 0:2], sb["cc"][:], AF.Copy)
            V.tensor_copy(sel3[:, :, 2], q[:])

            # d2-matmul lhsT quantities [-2x0, -2x1, 1, |x|^2] packed [P,T,4]
            prep4 = wtile("prep4", [P, T, 4])
            SC.activation(prep4[:, :, 0:2], sb["cc"][:], AF.Copy, scale=-2.0)
            V.memset(prep4[:, :, 2], 1.0)
            ccsq = wtile("ccsq", [P, T, 2])
            V.tensor_tensor(ccsq[:], sb["cc"][:], sb["cc"][:], OP.mult)
            V.tensor_tensor(prep4[:, :, 3], ccsq[:, :, 0], ccsq[:, :, 1],
                            OP.add)

            # extras: [noise*beta, noise, |x|^2, q] free-reduced to [P,4],
            # then partition-reduced to a [1,4] row via PE (ready for AR3)
            extras = io.tile([P, 4], f32, name="extras")
            nb_t = wtile("nb_t")
            V.tensor_tensor(nb_t[:], is_noise[:], beta[:], OP.mult)
            V.tensor_reduce(extras[:, 0:1], nb_t[:], mybir.AxisListType.X, OP.add)
            V.tensor_reduce(extras[:, 1:2], is_noise[:], mybir.AxisListType.X, OP.add)
            V.tensor_reduce(extras[:, 2:3], prep4[:, :, 3], mybir.AxisListType.X, OP.add)
            V.tensor_reduce(extras[:, 3:4], q[:], mybir.AxisListType.X, OP.add)
            extrasF = io.tile([1, 4], f32, name="extrasF")
            with tc.tile_pool(name="exp", bufs=1, space="PSUM") as exp_p:
                exPS = exp_p.tile([1, 4], f32, name="exPS")
                nc.tensor.matmul(exPS[:], onescol[:], extras[:],
                                 start=True, stop=True)
                SC.activation(extrasF[:], exPS[:], AF.Copy)

            # transpose prep4 -> lhsT4r [4, T, 128] (f32r, rounded at the
            # ACT evacuation so the fp32r d2 matmul accepts it)
            lhsT4r = io.tile([4, T, P], f32r, name="lhsT4r")
            for r in range(4):
                tp = psA.tile([P, P], f32, name="tpose4", tag="tpose")
                nc.tensor.transpose(tp[0:T, :], prep4[:, :, r], ident[:])
                stage = io.tile([T, P], f32r, name=f"tstage{r}")
                SC.activation(stage[:], tp[0:T, :], AF.Copy)
                nc.sync.dma_start(lhsT4r[r:r + 1, :, :], stage[:])

            # ---------- P2: local per-object max of (beta+1)-weighted one-hot
            # (DVE builds; bm[p,k] = (iota[k]==tidx[p,t]) * (beta[p,t]+1)) ----
            runmax = io.tile([P, K], f32, name="runmax")
            V.memset(runmax[:], 0.0)
            with tc.tile_pool(name="bmp", bufs=3) as bmp:
                for t in range(T):
                    bm = bmp.tile([P, K], f32, name="bm")
                    V.tensor_scalar(bm[:], iotaF[:], sb["tidx"][:, t:t + 1],
                                    betap1[:, t:t + 1], OP.is_equal, OP.mult)
                    V.tensor_tensor(runmax[:], runmax[:], bm[:], OP.max)

            # partition-reduce runmax -> Bloc [128,4] (k = 128*b + p)
            Bloc = io.tile([P, KB], f32, name="Bloc")
            for b in range(KB):
                tp = psA.tile([P, P], f32, name="tpose", tag="tpose")
                nc.tensor.transpose(tp[:], runmax[:, b * P:(b + 1) * P], ident[:])
                V.reduce_max(Bloc[:, b:b + 1], tp[:], axis=mybir.AxisListType.X)

            # row layout: BlocF[0, 128*b+p] = Bloc[p, b]
            BlocF = io.tile([1, K], f32, name="BlocF")
            for b in range(KB):
                nc.sync.dma_start(BlocF[0:1, b * P:(b + 1) * P], Bloc[:, b:b + 1])

            # ---------- P4a: AllReduce-max of BlocF (overlaps with P3) -------
            arm_in = dram.tile([1, K], f32, name="arm_in")
            arm_out = dram.tile([1, K], f32, name="arm_out", addr_space="Shared")
            nc.sync.dma_start(arm_in[0:1, :], BlocF[:])
            if cc_mode in ('all', 'first', 'two'):
                nc.gpsimd.collective_compute(
                    "AllReduce", OP.max,
                    replica_groups=[list(range(NCORES))],
                    ins=[arm_in[:]], outs=[arm_out[:]],
                )
            else:
                nc.sync.dma_start(arm_out[:], arm_in[:])
            BglobF = io.tile([1, K], f32, name="BglobF")
            nc.sync.dma_start(BglobF[:], arm_out[0:1, :])

            # broadcast BlocF across partitions via PE: ones[1,P].T @ BlocF
            BlocB = io.tile([P, K], f32, name="BlocB")
            with tc.tile_pool(name="bcp", bufs=1, space="PSUM") as bcp:
                blocps = bcp.tile([P, K], f32, name="blocps")
                nc.tensor.matmul(blocps[:], onesrow[:], BlocF[:],
                                 start=True, stop=True)
                SC.activation(BlocB[:], blocps[:], AF.Copy)

            # ---------- P3: selection segment-sums -> selPT rows [3, K] ------
            with (
                tc.tile_pool(name="selpp", bufs=1, space="PSUM") as selpp,
                tc.tile_pool(name="bmp3", bufs=3) as bmp3,
            ):
                selPT = selpp.tile([3, K], f32, name="selPT")
                V.memset(selPT[:], 0.0)
                for t in range(T):
                    bm = bmp3.tile([P, K], f32, name="bm3")
                    V.tensor_scalar(bm[:], iotaF[:], sb["tidx"][:, t:t + 1],
                                    betap1[:, t:t + 1], OP.is_equal, OP.mult)
                    Isel = bmp3.tile([P, K], f32r, name="Isel")
                    V.tensor_tensor(Isel[:], bm[:], BlocB[:], OP.is_equal)
                    nc.tensor.matmul(selPT[:], sel3[:, t, :], Isel[:],
                                     start=False, stop=(t == T - 1),
                                     skip_group_check=True)
                selsbT = io.tile([3, K], f32, name="selsbT")
                SC.activation(selsbT[:], selPT[:], AF.Copy)

            # gate by global-winner mask and AllReduce-add.  Compute engines
            # must start at partition 0/32/64/96, so replicate keepF to 3
            # partitions via DMA and gate with one [3,K] multiply.
            keepF = io.tile([1, K], f32, name="keepF")
            V.tensor_tensor(keepF[:], BlocF[:], BglobF[:], OP.is_equal)
            keep3 = io.tile([3, K], f32, name="keep3")
            for r in range(3):
                nc.sync.dma_start(keep3[r:r + 1, :], keepF[:])
            sel_cT = io.tile([3, K], f32, name="sel_cT")
            V.tensor_tensor(sel_cT[:], selsbT[:], keep3[:], OP.mult)
            ar2_in = dram.tile([1, 3 * K], f32, name="ar2_in")
            ar2_out = dram.tile([1, 3 * K], f32, name="ar2_out",
                                addr_space="Shared")
            nc.sync.dma_start(ar2_in[0:1, :], sel_cT[:])   # row-major pack
            if cc_mode in ('all', 'two'):
                nc.gpsimd.collective_compute(
                    "AllReduce", OP.add,
                    replica_groups=[list(range(NCORES))],
                    ins=[ar2_in[:]], outs=[ar2_out[:]],
                )
            else:
                nc.sync.dma_start(ar2_out[:], ar2_in[:])

            # rhsD2 rows: [xa0; xa1; |xa|^2; 1].  |xa|^2 is computed on
            # partition 0 (xa0F/xa1F row tiles) and DMA'd into row 2.
            rhsD2 = io.tile([4, K], f32, name="rhsD2")
            V.memset(rhsD2[:], 1.0)
            nc.sync.dma_start(
                rhsD2[0:2, :],
                ar2_out[0:1, 0:2 * K].rearrange("o (r k) -> (o r) k", r=2))
            xa0F = io.tile([1, K], f32, name="xa0F")
            nc.sync.dma_start(xa0F[:], ar2_out[0:1, 0:K])
            xa1F = io.tile([1, K], f32, name="xa1F")
            nc.sync.dma_start(xa1F[:], ar2_out[0:1, K:2 * K])
            qaF = io.tile([1, K], f32, name="qaF")
            nc.sync.dma_start(qaF[:], ar2_out[0:1, 2 * K:3 * K])
            xsqF = io.tile([1, K], f32, name="xsqF")
            xsq_t = io.tile([1, K], f32, name="xsq_t")
            V.tensor_tensor(xsq_t[:], xa1F[:], xa1F[:], OP.mult)
            V.tensor_tensor(xsqF[:], xa0F[:], xa0F[:], OP.mult)
            V.tensor_tensor(xsqF[:], xsqF[:], xsq_t[:], OP.add)
            nc.sync.dma_start(rhsD2[2:3, :], xsqF[:])
            rhsD2r = io.tile([4, K], f32r, name="rhsD2r")
            V.tensor_copy(rhsD2r[:], rhsD2[:])

            # prebuild the first NPRE segment one-hots around the AR2 window
            NPRE = 32
            bm6pre = io.tile([P, NPRE, K], f16, name="bm6pre")
            for t in range(NPRE):
                V.tensor_scalar(bm6pre[:, t, :], iotaH[:],
                                sb["tidx"][:, t:t + 1],
                                betap1[:, t:t + 1], OP.is_equal, OP.mult)

            qr = wtile("qr", dtype=f32r)      # rounded copy for fp32r matmul
            V.tensor_copy(qr[:], q[:])

            # ---------- P5 loop 1: d2 block, rep row-sums, self-distance -----
            # software-pipelined by one stage: d2 matmul for t+1 issues before
            # the rep matmul for t so the PE never blocks behind the sqrt.
            # rep accumulates q * relu(1 - s) directly (hinge on ACT).
            gstD = io.tile([P, T], f32, name="gstD")   # (beta+1)*s_self
            repP = accp.tile([1, K], f32, name="repP")
            V.memset(repP[:], 0.0)
            scr = io.tile([P, K], f16, name="scr")         # ttr full-out scratch
            with (
                tc.tile_pool(name="d2pool", bufs=3, space="PSUM") as d2pool,
                tc.tile_pool(name="sp", bufs=3) as sp,
                tc.tile_pool(name="bmp5", bufs=3) as bmp5,
            ):
                d2tiles = {}
                def d2mm(t):
                    d2ps = d2pool.tile([P, K], f32, name="d2ps")
                    nc.tensor.matmul(d2ps[:], lhsT4r[0:4, t, :], rhsD2r[:],
                                     start=True, stop=True)
                    d2tiles[t] = d2ps
                d2mm(0)
                for t in range(T):
                    if t + 1 < T:
                        d2mm(t + 1)
                    d2ps = d2tiles.pop(t)
                    bm = bmp5.tile([P, K], f16, name="bm5")
                    V.tensor_scalar(bm[:], iotaH[:], sb["tidx"][:, t:t + 1],
                                    betap1[:, t:t + 1], OP.is_equal, OP.mult)
                    sS = sp.tile([P, K], f16, name="sS")
                    SC.activation(sS[:], d2ps[:], AF.Sqrt, bias=cbias(SQ_BIAS))
                    rlu = sp.tile([P, K], f32r, name="rlu")
                    SC.activation(rlu[:], sS[:], AF.Relu, bias=cbias(1.0),
                                  scale=-1.0)
                    nc.tensor.matmul(repP[:], qr[:, t:t + 1], rlu[:],
                                     start=False, stop=(t == T - 1),
                                     skip_group_check=True)
                    V.scalar_tensor_tensor(
                        scr[:], bm[:], 1.0, sS[:], OP.bypass, OP.mult,
                        accum_out=gstD[:, t:t + 1])

            # ---------- global per-hit math for segment rhs ----------
            qrb = wtile("qrb")
            V.tensor_tensor(qrb[:], q[:], rb1[:], OP.mult)
            sself = wtile("sself")              # sqrt(d2_self + SQ_BIAS)
            V.tensor_tensor(sself[:], gstD[:], rb1[:], OP.mult)
            G2 = wtile("G2")                    # d2_self
            V.tensor_tensor(G2[:], sself[:], sself[:], OP.mult)
            V.tensor_scalar(G2[:], G2[:], SQ_BIAS, None, OP.subtract)
            s2 = wtile("s2")                    # min(s_self, 1)
            V.tensor_scalar(s2[:], sself[:], 1.0, None, OP.min)
            rhs_seg = io.tile([P, T, 6], f16, name="rhs_seg")
            # att' = q*d2_self/(b+1)
            V.tensor_tensor(rhs_seg[:, :, 0], G2[:], qrb[:], OP.mult)
            # qmin' = q*min(s_self,1)/(b+1)
            V.tensor_tensor(rhs_seg[:, :, 1], s2[:], qrb[:], OP.mult)
            V.tensor_tensor(rhs_seg[:, :, 2], sb["valid"][:], rb1[:], OP.mult)
            V.tensor_tensor(rhs_seg[:, :, 3], pw[:], rb1[:], OP.mult)
            V.tensor_tensor(rhs_seg[:, :, 4], paypw[:], rb1[:], OP.mult)
            V.tensor_copy(rhs_seg[:, :, 5], qrb[:])

            # ---------- P5 loop 2: segment sums -> segPT rows [6, K] ---------
            segPT = accp.tile([6, K], f32, name="segPT")
            V.memset(segPT[:], 0.0)
            with tc.tile_pool(name="bmp6", bufs=3) as bmp6:
                for t in range(T):
                    if t < NPRE:
                        bmap = bm6pre[:, t, :]
                    else:
                        bm = bmp6.tile([P, K], f16, name="bm6")
                        V.tensor_scalar(bm[:], iotaH[:],
                                        sb["tidx"][:, t:t + 1],
                                        betap1[:, t:t + 1],
                                        OP.is_equal, OP.mult)
                        bmap = bm[:]
                    nc.tensor.matmul(segPT[:], rhs_seg[:, t, :], bmap,
                                     start=False, stop=(t == T - 1),
                                     skip_group_check=True)

            # ---------- P6: AllReduce of per-object rows ----------
            segsbT = io.tile([6, K], f32, name="segsbT")
            SC.activation(segsbT[:], segPT[:], AF.Copy)
            repsb = io.tile([1, K], f32, name="repsb")
            SC.activation(repsb[:], repP[:], AF.Copy)

            NSEG = 6 * K
            NTOT = NSEG + K + 4
            ar_in = dram.tile([1, NTOT], f32, name="ar_in")
            ar_out = dram.tile([1, NTOT], f32, name="ar_out", addr_space="Shared")
            nc.sync.dma_start(ar_in[0:1, 0:NSEG], segsbT[:])     # row-major
            nc.sync.dma_start(ar_in[0:1, NSEG:NSEG + K], repsb[:])
            nc.sync.dma_start(ar_in[0:1, NSEG + K:NTOT], extrasF[:])
            if cc_mode == 'all':
                nc.gpsimd.collective_compute(
                    "AllReduce", OP.add,
                    replica_groups=[list(range(NCORES))],
                    ins=[ar_in[:]], outs=[ar_out[:]],
                )
            else:
                nc.sync.dma_start(ar_out[:], ar_in[:])
            # unpack per-object rows (partition-0 tiles; compute engines
            # cannot start at partitions 1..5)
            seg_rows = []
            for r in range(6):
                rt = io.tile([1, K], f32, name=f"segrow{r}")
                nc.sync.dma_start(rt[:], ar_out[0:1, r * K:(r + 1) * K])
                seg_rows.append(rt)
            repF = io.tile([1, K], f32, name="repF")
            nc.sync.dma_start(repF[:], ar_out[0:1, NSEG:NSEG + K])
            extras_g = io.tile([1, 4], f32, name="extras_g")
            nc.sync.dma_start(extras_g[:], ar_out[0:1, NSEG + K:NTOT])

            # ---------- P7: assembly ([1,K] rows) ----------
            attseg = seg_rows[0][:]
            qminseg = seg_rows[1][:]
            count = seg_rows[2][:]
            pwseg = seg_rows[3][:]
            payseg = seg_rows[4][:]
            qseg = seg_rows[5][:]

            def rtile(name):
                return io.tile([1, K], f32, name=name)

            has = rtile("has")
            V.tensor_scalar(has[:], count, 0.0, None, OP.is_gt)
            rc = rtile("rc")        # 1/(count+eps)
            V.tensor_scalar(rc[:], count, EPS, None, OP.add)
            V.reciprocal(rc[:], rc[:])
            rnc = rtile("rnc")      # 1/(N-count+eps)
            V.tensor_scalar(rnc[:], count, -1.0, float(N) + EPS,
                            OP.mult, OP.add)
            V.reciprocal(rnc[:], rnc[:])
            lpd = rtile("lpd")      # 1/(pwseg+eps)
            V.tensor_scalar(lpd[:], pwseg, EPS, None, OP.add)
            V.reciprocal(lpd[:], lpd[:])

            la = rtile("la")        # qa*attseg/(count+eps) * has
            V.tensor_tensor(la[:], attseg, qaF[:], OP.mult)
            V.tensor_tensor(la[:], la[:], rc[:], OP.mult)
            V.tensor_tensor(la[:], la[:], has[:], OP.mult)

            # rep_k = (repF - qseg + qminseg)*qa*rnc*has   (relu-form)
            lr = rtile("lr")
            V.tensor_tensor(lr[:], repF[:], qminseg, OP.add)
            V.tensor_tensor(lr[:], lr[:], qseg, OP.subtract)
            V.tensor_tensor(lr[:], lr[:], qaF[:], OP.mult)
            V.tensor_tensor(lr[:], lr[:], rnc[:], OP.mult)
            V.tensor_tensor(lr[:], lr[:], has[:], OP.mult)

            lb = rtile("lb")        # has*(1 - beta_alpha) = has*(2 - Bglob)
            V.tensor_scalar(lb[:], BglobF[:], -1.0, 2.0, OP.mult, OP.add)
            V.tensor_tensor(lb[:], lb[:], has[:], OP.mult)

            lp = rtile("lp")        # has*paynum/(payden+eps)
            V.tensor_tensor(lp[:], lpd[:], payseg, OP.mult)
            V.tensor_tensor(lp[:], lp[:], has[:], OP.mult)

            lsum = rtile("lsum")
            V.tensor_tensor(lsum[:], la[:], lr[:], OP.add)
            V.tensor_tensor(lsum[:], lsum[:], lb[:], OP.add)
            V.tensor_tensor(lsum[:], lsum[:], lp[:], OP.add)
            fin = io.tile([1, 2], f32, name="fin")
            V.tensor_reduce(fin[0:1, 0:1], lsum[:], mybir.AxisListType.X, OP.add)
            V.tensor_reduce(fin[0:1, 1:2], has[:], mybir.AxisListType.X, OP.add)

            # total = lsum/n_obj + nb/(nn+eps) + 0.001*xsq/(2N)
            nobj = io.tile([1, 1], f32, name="nobj")
            V.tensor_scalar(nobj[:], fin[0:1, 1:2], EPS, None, OP.add)
            V.reciprocal(nobj[:], nobj[:])
            tot = io.tile([1, 1], f32, name="tot")
            V.tensor_tensor(tot[:], fin[0:1, 0:1], nobj[:], OP.mult)
            nden = io.tile([1, 1], f32, name="nden")
            V.tensor_scalar(nden[:], extras_g[0:1, 1:2], EPS, None, OP.add)
            V.reciprocal(nden[:], nden[:])
            V.tensor_tensor(nden[:], nden[:], extras_g[0:1, 0:1], OP.mult)
            V.tensor_tensor(tot[:], tot[:], nden[:], OP.add)
            lcc = io.tile([1, 1], f32, name="lcc")
            SC.activation(lcc[:], extras_g[0:1, 2:3], AF.Copy,
                          scale=0.001 / (2.0 * N))
            V.tensor_tensor(tot[:], tot[:], lcc[:], OP.add)
            nc.sync.dma_start(out_d.ap(), tot[:])

    nc.compile()
    return nc


def _host_prep(inputs):
    """Slice, pad and re-layout the full inputs into 8 per-core input maps."""
    def lay(a2):                       # [SP, w] -> [128, T, w]
        w = a2.shape[1]
        r = a2.reshape(T, P, w).transpose(1, 0, 2)
        return np.ascontiguousarray(r.astype(np.float32))

    in_maps = []
    for c in range(NCORES):
        sl = slice(c * S, (c + 1) * S)

        def pad(a, fill=0.0):
            out = np.full((SP, a.shape[1]), fill, np.float32)
            out[:S] = a[sl]
            return out

        tidx = np.full((SP, 1), -2.0, np.float32)
        tidx[:S, 0] = inputs["t_idx"][sl, 0].astype(np.float32)
        valid = np.zeros((SP, 1), np.float32)
        valid[:S] = 1.0
        m = {
            "beta_r": lay(pad(inputs["pred_beta"]))[:, :, 0],
            "cc": lay(pad(inputs["pred_ccoords"])),
            "pE": lay(pad(inputs["pred_energy"]))[:, :, 0],
            "ppos": lay(pad(inputs["pred_pos"])),
            "ptime": lay(pad(inputs["pred_time"]))[:, :, 0],
            "pid": lay(pad(inputs["pred_id"])),
            "tE": lay(pad(inputs["t_energy"]))[:, :, 0],
            "tpos": lay(pad(inputs["t_pos"])),
            "ttime": lay(pad(inputs["t_time"]))[:, :, 0],
            "tidx": lay(tidx)[:, :, 0],
            "valid": lay(valid)[:, :, 0],
        }
        m = {k: np.ascontiguousarray(v) for k, v in m.items()}
        in_maps.append(m)
    return in_maps


def _run(inputs, trace=False):
    from concourse import bass_utils
    if "nc" not in _CACHE:
        _CACHE["nc"] = _build()
    nc = _CACHE["nc"]
    in_maps = _host_prep(inputs)
    res = bass_utils.run_bass_kernel_spmd(
        nc, in_maps, core_ids=list(range(NCORES)), trace=trace)
    return res


def kernel(**inputs):
    res = _run(inputs, trace=False)
    val = np.float32(res.results[0]["out"][0, 0])
    return np.array(val, dtype=np.float32)[()]


if __name__ == "__main__":
    d = np.load("/tmp/inputs.npz")
    inp = {k: d[k] for k in d.files}
    print("kernel:", kernel(**inp))


# revision 32
# speedup vs baseline: 1.1368x; 1.0336x over previous
"""Trainium2 Bass kernel for LLFullObjectCondensation loss (N=80000, K=512, C=2).

Strategy (8 NeuronCores, data-parallel over hits):
  - Each core gets a 10000-hit shard (padded to 79*128=10112), laid out [128, 79].
  - P1: per-hit quantities (q, payload, weights) as full-width [128,79] ops.
  - P2: per-object max of (beta+1)-weighted one-hot tiles (DVE build + running
        max), 4 PE transposes + reductions -> Bloc, packed to row BlocF [1,K].
  - AllReduce-max of BlocF; P3 selection pass overlaps: Isel = (bm == BlocB),
        PE matmul with sel3 stationary -> selPT [3,K] rows, gated by
        keepF = (BlocF == BglobF), AllReduce-add -> global x_alpha/q_alpha rows.
  - P5 loop 1 (software-pipelined): d2 via PE matmul (contract-4 trick), sqrt
        on ACT, min on DVE, repulsion row-sums via PE, self-distance gather via
        DVE row-reduce with the weighted one-hot.
  - P5 loop 2: segment sums as PE matmuls, rhs_seg [128,6] stationary, bm
        moving -> segPT [6,K] rows.
  - AllReduce-add of all per-object rows, then row-layout [1,K] assembly of
        the scalar loss.
All one-hot builds and elementwise work run on DVE/ACT (GPSIMD's software
tensor_scalar measured ~8us per [128,512] tile vs DVE's ~0.55us).
"""
import sys
import numpy as np

for _p in ("/opt/trn_rl_repo", "/root/.axon_site/_ro/trn_rl_repo"):
    if _p not in sys.path:
        sys.path.append(_p)

N = 80000
K = 512
NCORES = 8
S = N // NCORES          # 10000 hits per core
P = 128
T = 79                   # tiles per core, T*P = 10112 >= S
SP = T * P
KB = K // P              # 4 k-blocks
EPS = 1e-9
SQ_BIAS = 2e-2           # reference uses 1e-6; extra margin absorbs fp32r
                         # matmul rounding of the expanded |x|^2-2x.a+|a|^2
                         # form so sqrt never sees a negative input (measured:
                         # 1e-3 still went NaN -> fp32r product error ~5e-3 abs
                         # on O(20) terms; the 0.02 bias shifts the hinge by
                         # ~1.2e-3 relative on the total loss, gate is 2e-2)

_CACHE = {}


def _build(cc_mode='all'):
    import concourse.bass as bass
    import concourse.bacc as bacc
    import concourse.mybir as mybir
    import concourse.tile as tile
    from concourse import masks

    f32 = mybir.dt.float32
    f32r = mybir.dt.float32r
    i32 = mybir.dt.int32
    u16 = mybir.dt.uint16
    f16 = mybir.dt.float16
    AF = mybir.ActivationFunctionType
    OP = mybir.AluOpType

    nc = bacc.Bacc("TRN2", target_bir_lowering=False, debug=False,
                   num_devices=NCORES)

    di = {}
    def din(name, shape):
        di[name] = nc.dram_tensor(name, shape, f32, kind="ExternalInput")
        return di[name]

    din("beta_r", [P, T])
    din("cc", [P, T, 2])
    din("pE", [P, T])
    din("ppos", [P, T, 2])
    din("ptime", [P, T])
    din("pid", [P, T, 6])
    din("tE", [P, T])
    din("tpos", [P, T, 2])
    din("ttime", [P, T])
    din("tidx", [P, T])
    din("valid", [P, T])
    out_d = nc.dram_tensor("out", [1, 1], f32, kind="ExternalOutput")

    with tile.TileContext(nc) as tc:
        with (
            tc.tile_pool(name="const", bufs=1) as cpool,
            tc.tile_pool(name="io", bufs=1) as io,
            tc.tile_pool(name="dram", bufs=1, space="DRAM") as dram,
            tc.tile_pool(name="psA", bufs=2, space="PSUM") as psA,
            tc.tile_pool(name="acc", bufs=1, space="PSUM") as accp,
        ):
            # ---------- constants ----------
            ident = cpool.tile([P, P], f32)
            masks.make_identity(nc, ident[:])
            iotaI = cpool.tile([P, K], i32)
            nc.gpsimd.iota(iotaI[:], pattern=[[1, K]], base=0,
                           channel_multiplier=0)
            iotaF = cpool.tile([P, K], f32)
            nc.vector.tensor_copy(iotaF[:], iotaI[:])
            iotaH = cpool.tile([P, K], f16)
            nc.vector.tensor_copy(iotaH[:], iotaI[:])
            onescol = cpool.tile([P, 1], f32)
            nc.vector.memset(onescol[:], 1.0)
            onesrow = cpool.tile([1, P], f32)
            nc.vector.memset(onesrow[:], 1.0)

            _cb = {}
            def cbias(val):
                """[128,1] constant column for activation bias operands."""
                if val not in _cb:
                    ct = cpool.tile([P, 1], f32, name=f"cb{len(_cb)}")
                    nc.vector.memset(ct[:], val)
                    _cb[val] = ct
                return _cb[val][:]

            # ---------- load inputs ----------
            sb = {}
            for name, h in di.items():
                t_sb = io.tile(list(h.shape), f32, name=f"sb_{name}")
                nc.sync.dma_start(t_sb[:], h.ap())
                sb[name] = t_sb

            # ---------- P1: per-hit prep (all [128,T]-wide ops) ----------
            V = nc.vector
            SC = nc.scalar

            def wtile(name, shape=None, dtype=None):
                return io.tile(shape or [P, T], dtype or f32, name=name)
            u8 = mybir.dt.uint8

            beta = wtile("beta")
            V.tensor_scalar(beta[:], sb["beta_r"][:], 1e-6, 1.0 - 1e-6,
                            OP.max, OP.min)
            betap1 = wtile("betap1")
            SC.activation(betap1[:], beta[:], AF.Identity, bias=cbias(1.0))
            rb1 = wtile("rb1")
            V.reciprocal(rb1[:], betap1[:])
            onem = wtile("onem")
            SC.activation(onem[:], beta[:], AF.Identity, bias=cbias(1.0), scale=-1.0)
            recm = wtile("recm")
            V.reciprocal(recm[:], onem[:])
            ratio = wtile("ratio")
            V.tensor_tensor(ratio[:], betap1[:], recm[:], OP.mult)

            is_noise = wtile("is_noise")
            V.tensor_scalar(is_noise[:], sb["tidx"][:], -1.0, None, OP.is_equal)
            is_obj = wtile("is_obj")
            V.tensor_scalar(is_obj[:], sb["tidx"][:], 0.0, None, OP.is_ge)

            # energy weights w = relu(min(wr,1)) ; wr=(tE-0.5)/9.5
            wr = wtile("wr")
            SC.activation(wr[:], sb["tE"][:], AF.Identity, bias=cbias(-0.5 / 9.5),
                          scale=1.0 / 9.5)
            ew = wtile("ew")
            V.tensor_scalar(ew[:], wr[:], 1.0, 0.0, OP.min, OP.max)
            pw = wtile("pw")
            V.tensor_tensor(pw[:], beta[:], ew[:], OP.mult)
            V.tensor_tensor(pw[:], pw[:], is_obj[:], OP.mult)

            # --- energy loss pieces (pre-transcendental) ---
            ediff_r = wtile("ediff_r")
            V.tensor_tensor(ediff_r[:], sb["tE"][:], sb["pE"][:], OP.subtract)
            ediff = wtile("ediff")
            SC.activation(ediff[:], ediff_r[:], AF.Abs)
            ed2 = wtile("ed2")
            V.tensor_tensor(ed2[:], ediff[:], ediff[:], OP.mult)
            ed001 = wtile("ed001")
            SC.activation(ed001[:], ediff[:], AF.Copy, scale=0.001)

            # --- position loss pieces ---
            dpos = wtile("dpos", [P, T, 2])
            V.tensor_tensor(dpos[:], sb["tpos"][:], sb["ppos"][:], OP.subtract)
            V.tensor_tensor(dpos[:], dpos[:], dpos[:], OP.mult)
            d2p = wtile("d2p")
            V.tensor_tensor(d2p[:], dpos[:, :, 0], dpos[:, :, 1], OP.add)

            # --- timing loss pieces ---
            dtm = wtile("dtm")
            V.tensor_tensor(dtm[:], sb["ttime"][:], sb["ptime"][:], OP.subtract)
            adt = wtile("adt")
            SC.activation(adt[:], dtm[:], AF.Abs)
            dt2 = wtile("dt2")
            V.tensor_tensor(dt2[:], dtm[:], dtm[:], OP.mult)
            lint = wtile("lint")
            SC.activation(lint[:], adt[:], AF.Identity, bias=cbias(-4.0), scale=4.0)
            ltt = wtile("ltt", dtype=u8)
            V.tensor_scalar(ltt[:], adt[:], 2.0, None, OP.is_lt)
            ht = wtile("ht")
            V.select(ht[:], ltt[:], dt2[:], lint[:])
            yt = wtile("yt")
            SC.activation(yt[:], ht[:], AF.Copy, scale=1.0 / 6.0)

            # --- classification loss ---
            pid2 = wtile("pid2", [P, T, 6])
            V.tensor_tensor(pid2[:], sb["pid"][:], sb["pid"][:], OP.mult)
            cred = wtile("cred")
            V.tensor_reduce(cred[:], pid2[:], mybir.AxisListType.X, OP.add)

            # --- transcendental block: Exp, then Sqrt, then Ln (grouped to
            # limit ACT table swaps) ---
            ex = wtile("ex")
            SC.activation(ex[:], ed2[:], AF.Exp, scale=-0.1)
            xp = wtile("xp")
            SC.activation(xp[:], d2p[:], AF.Sqrt, bias=cbias(0.01), scale=0.01)

            lnr = wtile("lnr")
            SC.activation(lnr[:], ratio[:], AF.Ln)
            # q = (0.5*ln(ratio))^2 + 0.1, zeroed on padding
            halfln = wtile("halfln")
            SC.activation(halfln[:], lnr[:], AF.Copy, scale=0.5)
            q = wtile("q")
            V.tensor_tensor(q[:], halfln[:], halfln[:], OP.mult)
            V.scalar_tensor_tensor(q[:], q[:], 0.1, sb["valid"][:],
                                   OP.add, OP.mult)

            # energy softclip
            ye = wtile("ye")
            V.tensor_tensor(ye[:], ex[:], ed001[:], OP.add)
            lnye = wtile("lnye")
            SC.activation(lnye[:], ye[:], AF.Ln, bias=cbias(1.0))
            gte = wtile("gte", dtype=u8)
            V.tensor_scalar(gte[:], ye[:], 1.0, None, OP.is_gt)
            esc = wtile("esc")
            V.select(esc[:], gte[:], lnye[:], ye[:])

            # position huber + softclip
            xp2 = wtile("xp2")
            V.tensor_tensor(xp2[:], xp[:], xp[:], OP.mult)
            linp = wtile("linp")
            SC.activation(linp[:], xp[:], AF.Identity, bias=cbias(-100.0), scale=20.0)
            ltp = wtile("ltp", dtype=u8)
            V.tensor_scalar(ltp[:], xp[:], 10.0, None, OP.is_lt)
            hp = wtile("hp")
            V.select(hp[:], ltp[:], xp2[:], linp[:])
            yp = wtile("yp")
            SC.activation(yp[:], hp[:], AF.Copy, scale=1.0 / 3.0)
            lnyp = wtile("lnyp")
            SC.activation(lnyp[:], yp[:], AF.Ln, bias=cbias(1.0))
            gtp = wtile("gtp", dtype=u8)
            V.tensor_scalar(gtp[:], yp[:], 1.0, None, OP.is_gt)
            psc = wtile("psc")
            V.select(psc[:], gtp[:], lnyp[:], yp[:])

            # timing softclip
            lnyt = wtile("lnyt")
            SC.activation(lnyt[:], yt[:], AF.Ln, bias=cbias(1.0))
            gtt = wtile("gtt", dtype=u8)
            V.tensor_scalar(gtt[:], yt[:], 1.0, None, OP.is_gt)
            tsc = wtile("tsc")
            V.select(tsc[:], gtt[:], lnyt[:], yt[:])

            # payload = 10*esc + 3*psc + 6*tsc + (1e-8/6)*cred
            esc10 = wtile("esc10")
            SC.activation(esc10[:], esc[:], AF.Copy, scale=10.0)
            pay = wtile("pay")
            V.scalar_tensor_tensor(pay[:], psc[:], 3.0, esc10[:],
                                   OP.mult, OP.add)
            V.scalar_tensor_tensor(pay[:], tsc[:], 6.0, pay[:],
                                   OP.mult, OP.add)
            V.scalar_tensor_tensor(pay[:], cred[:], 1e-8 / 6.0, pay[:],
                                   OP.mult, OP.add)
            paypw = wtile("paypw")
            V.tensor_tensor(paypw[:], pay[:], pw[:], OP.mult)

            # selection rhs: [x0, x1, q] (stationary operand for P3 matmuls)
            sel3 = wtile("sel3", [P, T, 3], dtype=f32r)
            SC.activation(sel3[:, :, 0:2], sb["cc"][:], AF.Copy)
            V.tensor_copy(sel3[:, :, 2], q[:])

            # d2-matmul lhsT quantities [-2x0, -2x1, 1, |x|^2] packed [P,T,4]
            prep4 = wtile("prep4", [P, T, 4])
            SC.activation(prep4[:, :, 0:2], sb["cc"][:], AF.Copy, scale=-2.0)
            V.memset(prep4[:, :, 2], 1.0)
            ccsq = wtile("ccsq", [P, T, 2])
            V.tensor_tensor(ccsq[:], sb["cc"][:], sb["cc"][:], OP.mult)
            V.tensor_tensor(prep4[:, :, 3], ccsq[:, :, 0], ccsq[:, :, 1],
                            OP.add)

            # extras: [noise*beta, noise, |x|^2, q] free-reduced to [P,4],
            # then partition-reduced to a [1,4] row via PE (ready for AR3)
            extras = io.tile([P, 4], f32, name="extras")
            nb_t = wtile("nb_t")
            V.tensor_tensor(nb_t[:], is_noise[:], beta[:], OP.mult)
            V.tensor_reduce(extras[:, 0:1], nb_t[:], mybir.AxisListType.X, OP.add)
            V.tensor_reduce(extras[:, 1:2], is_noise[:], mybir.AxisListType.X, OP.add)
            V.tensor_reduce(extras[:, 2:3], prep4[:, :, 3], mybir.AxisListType.X, OP.add)
            V.tensor_reduce(extras[:, 3:4], q[:], mybir.AxisListType.X, OP.add)
            extrasF = io.tile([1, 4], f32, name="extrasF")
            with tc.tile_pool(name="exp", bufs=1, space="PSUM") as exp_p:
                exPS = exp_p.tile([1, 4], f32, name="exPS")
                nc.tensor.matmul(exPS[:], onescol[:], extras[:],
                                 start=True, stop=True)
                SC.activation(extrasF[:], exPS[:], AF.Copy)

            # transpose prep4 -> lhsT4r [4, T, 128] (f32r, rounded at the
            # ACT evacuation so the fp32r d2 matmul accepts it)
            lhsT4r = io.tile([4, T, P], f32r, name="lhsT4r")
            for r in range(4):
                tp = psA.tile([P, P], f32, name="tpose4", tag="tpose")
                nc.tensor.transpose(tp[0:T, :], prep4[:, :, r], ident[:])
                stage = io.tile([T, P], f32r, name=f"tstage{r}")
                SC.activation(stage[:], tp[0:T, :], AF.Copy)
                nc.sync.dma_start(lhsT4r[r:r + 1, :, :], stage[:])

            # ---------- P2: local per-object max of (beta+1)-weighted one-hot
            # (DVE builds; bm[p,k] = (iota[k]==tidx[p,t]) * (beta[p,t]+1)) ----
            runmax = io.tile([P, K], f32, name="runmax")
            V.memset(runmax[:], 0.0)
            with tc.tile_pool(name="bmp", bufs=3) as bmp:
                for t in range(T):
                    bm = bmp.tile([P, K], f32, name="bm")
                    V.tensor_scalar(bm[:], iotaF[:], sb["tidx"][:, t:t + 1],
                                    betap1[:, t:t + 1], OP.is_equal, OP.mult)
                    V.tensor_tensor(runmax[:], runmax[:], bm[:], OP.max)

            # partition-reduce runmax -> Bloc [128,4] (k = 128*b + p)
            Bloc = io.tile([P, KB], f32, name="Bloc")
            for b in range(KB):
                tp = psA.tile([P, P], f32, name="tpose", tag="tpose")
                nc.tensor.transpose(tp[:], runmax[:, b * P:(b + 1) * P], ident[:])
                V.reduce_max(Bloc[:, b:b + 1], tp[:], axis=mybir.AxisListType.X)

            # row layout: BlocF[0, 128*b+p] = Bloc[p, b]
            BlocF = io.tile([1, K], f32, name="BlocF")
            for b in range(KB):
                nc.sync.dma_start(BlocF[0:1, b * P:(b + 1) * P], Bloc[:, b:b + 1])

            # ---------- P4a: AllReduce-max of BlocF (overlaps with P3) -------
            arm_in = dram.tile([1, K], f32, name="arm_in")
            arm_out = dram.tile([1, K], f32, name="arm_out", addr_space="Shared")
            nc.sync.dma_start(arm_in[0:1, :], BlocF[:])
            if cc_mode in ('all', 'first', 'two'):
                nc.gpsimd.collective_compute(
                    "AllReduce", OP.max,
                    replica_groups=[list(range(NCORES))],
                    ins=[arm_in[:]], outs=[arm_out[:]],
                )
            else:
                nc.sync.dma_start(arm_out[:], arm_in[:])
            BglobF = io.tile([1, K], f32, name="BglobF")
            nc.sync.dma_start(BglobF[:], arm_out[0:1, :])

            # broadcast BlocF across partitions via PE: ones[1,P].T @ BlocF
            BlocB = io.tile([P, K], f32, name="BlocB")
            with tc.tile_pool(name="bcp", bufs=1, space="PSUM") as bcp:
                blocps = bcp.tile([P, K], f32, name="blocps")
                nc.tensor.matmul(blocps[:], onesrow[:], BlocF[:],
                                 start=True, stop=True)
                SC.activation(BlocB[:], blocps[:], AF.Copy)

            # ---------- P3: selection segment-sums -> selPT rows [3, K] ------
            with (
                tc.tile_pool(name="selpp", bufs=1, space="PSUM") as selpp,
                tc.tile_pool(name="bmp3", bufs=3) as bmp3,
            ):
                selPT = selpp.tile([3, K], f32, name="selPT")
                V.memset(selPT[:], 0.0)
                for t in range(T):
                    bm = bmp3.tile([P, K], f32, name="bm3")
                    V.tensor_scalar(bm[:], iotaF[:], sb["tidx"][:, t:t + 1],
                                    betap1[:, t:t + 1], OP.is_equal, OP.mult)
                    Isel = bmp3.tile([P, K], f32r, name="Isel")
                    V.tensor_tensor(Isel[:], bm[:], BlocB[:], OP.is_equal)
                    nc.tensor.matmul(selPT[:], sel3[:, t, :], Isel[:],
                                     start=False, stop=(t == T - 1),
                                     skip_group_check=True)
                selsbT = io.tile([3, K], f32, name="selsbT")
                SC.activation(selsbT[:], selPT[:], AF.Copy)

            # gate by global-winner mask and AllReduce-add.  Compute engines
            # must start at partition 0/32/64/96, so replicate keepF to 3
            # partitions via DMA and gate with one [3,K] multiply.
            keepF = io.tile([1, K], f32, name="keepF")
            V.tensor_tensor(keepF[:], BlocF[:], BglobF[:], OP.is_equal)
            keep3 = io.tile([3, K], f32, name="keep3")
            for r in range(3):
                nc.sync.dma_start(keep3[r:r + 1, :], keepF[:])
            sel_cT = io.tile([3, K], f32, name="sel_cT")
            V.tensor_tensor(sel_cT[:], selsbT[:], keep3[:], OP.mult)
            ar2_in = dram.tile([1, 3 * K], f32, name="ar2_in")
            ar2_out = dram.tile([1, 3 * K], f32, name="ar2_out",
                                addr_space="Shared")
            nc.sync.dma_start(ar2_in[0:1, :], sel_cT[:])   # row-major pack
            if cc_mode in ('all', 'two'):
                nc.gpsimd.collective_compute(
                    "AllReduce", OP.add,
                    replica_groups=[list(range(NCORES))],
                    ins=[ar2_in[:]], outs=[ar2_out[:]],
                )
            else:
                nc.sync.dma_start(ar2_out[:], ar2_in[:])

            # rhsD2 rows: [xa0; xa1; |xa|^2; 1].  |xa|^2 is computed on
            # partition 0 (xa0F/xa1F row tiles) and DMA'd into row 2.
            rhsD2 = io.tile([4, K], f32, name="rhsD2")
            V.memset(rhsD2[:], 1.0)
            nc.sync.dma_start(
                rhsD2[0:2, :],
                ar2_out[0:1, 0:2 * K].rearrange("o (r k) -> (o r) k", r=2))
            xa0F = io.tile([1, K], f32, name="xa0F")
            nc.sync.dma_start(xa0F[:], ar2_out[0:1, 0:K])
            xa1F = io.tile([1, K], f32, name="xa1F")
            nc.sync.dma_start(xa1F[:], ar2_out[0:1, K:2 * K])
            qaF = io.tile([1, K], f32, name="qaF")
            nc.sync.dma_start(qaF[:], ar2_out[0:1, 2 * K:3 * K])
            xsqF = io.tile([1, K], f32, name="xsqF")
            xsq_t = io.tile([1, K], f32, name="xsq_t")
            V.tensor_tensor(xsq_t[:], xa1F[:], xa1F[:], OP.mult)
            V.tensor_tensor(xsqF[:], xa0F[:], xa0F[:], OP.mult)
            V.tensor_tensor(xsqF[:], xsqF[:], xsq_t[:], OP.add)
            nc.sync.dma_start(rhsD2[2:3, :], xsqF[:])
            rhsD2r = io.tile([4, K], f32r, name="rhsD2r")
            V.tensor_copy(rhsD2r[:], rhsD2[:])

            # prebuild the first NPRE segment one-hots around the AR2 window
            NPRE = 36
            bm6pre = io.tile([P, NPRE, K], f16, name="bm6pre")
            for t in range(NPRE):
                V.tensor_scalar(bm6pre[:, t, :], iotaH[:],
                                sb["tidx"][:, t:t + 1],
                                betap1[:, t:t + 1], OP.is_equal, OP.mult)

            qr = wtile("qr", dtype=f32r)      # rounded copy for fp32r matmul
            V.tensor_copy(qr[:], q[:])

            # ---------- P5 loop 1: d2 block, rep row-sums, self-distance -----
            # software-pipelined by one stage: d2 matmul for t+1 issues before
            # the rep matmul for t so the PE never blocks behind the sqrt.
            # rep accumulates q * relu(1 - s) directly (hinge on ACT).
            gstD = io.tile([P, T], f32, name="gstD")   # (beta+1)*s_self
            repP = accp.tile([1, K], f32, name="repP")
            V.memset(repP[:], 0.0)
            scr = io.tile([P, K], f16, name="scr")         # ttr full-out scratch
            with (
                tc.tile_pool(name="d2pool", bufs=3, space="PSUM") as d2pool,
                tc.tile_pool(name="sp", bufs=3) as sp,
                tc.tile_pool(name="bmp5", bufs=3) as bmp5,
            ):
                d2tiles = {}
                def d2mm(t):
                    d2ps = d2pool.tile([P, K], f32, name="d2ps")
                    nc.tensor.matmul(d2ps[:], lhsT4r[0:4, t, :], rhsD2r[:],
                                     start=True, stop=True)
                    d2tiles[t] = d2ps
                d2mm(0)
                sSp = None
                for t in range(T):
                    if t + 1 < T:
                        d2mm(t + 1)
                    d2ps = d2tiles.pop(t)
                    bm = bmp5.tile([P, K], f16, name="bm5")
                    V.tensor_scalar(bm[:], iotaH[:], sb["tidx"][:, t:t + 1],
                                    betap1[:, t:t + 1], OP.is_equal, OP.mult)
                    half = t % 2
                    if half == 0:
                        sSp = sp.tile([P, 2, K], f16, name="sSp")
                        rlup = sp.tile([P, 2, K], f32r, name="rlup")
                    sS = sSp[:, half, :]
                    SC.activation(sS, d2ps[:], AF.Sqrt, bias=cbias(SQ_BIAS))
                    # hinge for the pair in one ACT pass (halves the per-op
                    # overhead); last odd tile runs as a single
                    if half == 1:
                        SC.activation(rlup[:], sSp[:], AF.Relu,
                                      bias=cbias(1.0), scale=-1.0)
                        for h in (0, 1):
                            nc.tensor.matmul(repP[:], qr[:, t - 1 + h:t + h],
                                             rlup[:, h, :],
                                             start=False, stop=False,
                                             skip_group_check=True)
                    elif t == T - 1:
                        SC.activation(rlup[:, 0, :], sS, AF.Relu,
                                      bias=cbias(1.0), scale=-1.0)
                        nc.tensor.matmul(repP[:], qr[:, t:t + 1],
                                         rlup[:, 0, :],
                                         start=False, stop=True,
                                         skip_group_check=True)
                    V.scalar_tensor_tensor(
                        scr[:], bm[:], 1.0, sS, OP.bypass, OP.mult,
                        accum_out=gstD[:, t:t + 1])

            # ---------- global per-hit math for segment rhs ----------
            qrb = wtile("qrb")
            V.tensor_tensor(qrb[:], q[:], rb1[:], OP.mult)
            sself = wtile("sself")              # sqrt(d2_self + SQ_BIAS)
            V.tensor_tensor(sself[:], gstD[:], rb1[:], OP.mult)
            G2 = wtile("G2")                    # d2_self
            V.tensor_tensor(G2[:], sself[:], sself[:], OP.mult)
            V.tensor_scalar(G2[:], G2[:], SQ_BIAS, None, OP.subtract)
            s2 = wtile("s2")                    # min(s_self, 1)
            V.tensor_scalar(s2[:], sself[:], 1.0, None, OP.min)
            rhs_seg = io.tile([P, T, 6], f16, name="rhs_seg")
            # att' = q*d2_self/(b+1)
            V.tensor_tensor(rhs_seg[:, :, 0], G2[:], qrb[:], OP.mult)
            # qmin' = q*min(s_self,1)/(b+1)
            V.tensor_tensor(rhs_seg[:, :, 1], s2[:], qrb[:], OP.mult)
            V.tensor_tensor(rhs_seg[:, :, 2], sb["valid"][:], rb1[:], OP.mult)
            V.tensor_tensor(rhs_seg[:, :, 3], pw[:], rb1[:], OP.mult)
            V.tensor_tensor(rhs_seg[:, :, 4], paypw[:], rb1[:], OP.mult)
            V.tensor_copy(rhs_seg[:, :, 5], qrb[:])

            # ---------- P5 loop 2: segment sums -> segPT rows [6, K] ---------
            segPT = accp.tile([6, K], f32, name="segPT")
            V.memset(segPT[:], 0.0)
            with tc.tile_pool(name="bmp6", bufs=3) as bmp6:
                for t in range(T):
                    if t < NPRE:
                        bmap = bm6pre[:, t, :]
                    else:
                        bm = bmp6.tile([P, K], f16, name="bm6")
                        V.tensor_scalar(bm[:], iotaH[:],
                                        sb["tidx"][:, t:t + 1],
                                        betap1[:, t:t + 1],
                                        OP.is_equal, OP.mult)
                        bmap = bm[:]
                    nc.tensor.matmul(segPT[:], rhs_seg[:, t, :], bmap,
                                     start=False, stop=(t == T - 1),
                                     skip_group_check=True)

            # ---------- P6: AllReduce of per-object rows ----------
            segsbT = io.tile([6, K], f32, name="segsbT")
            SC.activation(segsbT[:], segPT[:], AF.Copy)
            repsb = io.tile([1, K], f32, name="repsb")
            SC.activation(repsb[:], repP[:], AF.Copy)

            NSEG = 6 * K
            NTOT = NSEG + K + 4
            ar_in = dram.tile([1, NTOT], f32, name="ar_in")
            ar_out = dram.tile([1, NTOT], f32, name="ar_out", addr_space="Shared")
            nc.sync.dma_start(ar_in[0:1, 0:NSEG], segsbT[:])     # row-major
            nc.sync.dma_start(ar_in[0:1, NSEG:NSEG + K], repsb[:])
            nc.sync.dma_start(ar_in[0:1, NSEG + K:NTOT], extrasF[:])
            if cc_mode == 'all':
                nc.gpsimd.collective_compute(
                    "AllReduce", OP.add,
                    replica_groups=[list(range(NCORES))],
                    ins=[ar_in[:]], outs=[ar_out[:]],
                )
            else:
                nc.sync.dma_start(ar_out[:], ar_in[:])
            # unpack per-object rows (partition-0 tiles; compute engines
            # cannot start at partitions 1..5)
            seg_rows = []
            for r in range(6):
                rt = io.tile([1, K], f32, name=f"segrow{r}")
                nc.sync.dma_start(rt[:], ar_out[0:1, r * K:(r + 1) * K])
                seg_rows.append(rt)
            repF = io.tile([1, K], f32, name="repF")
            nc.sync.dma_start(repF[:], ar_out[0:1, NSEG:NSEG + K])
            extras_g = io.tile([1, 4], f32, name="extras_g")
            nc.sync.dma_start(extras_g[:], ar_out[0:1, NSEG + K:NTOT])

            # ---------- P7: assembly ([1,K] rows) ----------
            attseg = seg_rows[0][:]
            qminseg = seg_rows[1][:]
            count = seg_rows[2][:]
            pwseg = seg_rows[3][:]
            payseg = seg_rows[4][:]
            qseg = seg_rows[5][:]

            def rtile(name):
                return io.tile([1, K], f32, name=name)

            has = rtile("has")
            V.tensor_scalar(has[:], count, 0.0, None, OP.is_gt)
            rc = rtile("rc")        # 1/(count+eps)
            V.tensor_scalar(rc[:], count, EPS, None, OP.add)
            V.reciprocal(rc[:], rc[:])
            rnc = rtile("rnc")      # 1/(N-count+eps)
            V.tensor_scalar(rnc[:], count, -1.0, float(N) + EPS,
                            OP.mult, OP.add)
            V.reciprocal(rnc[:], rnc[:])
            lpd = rtile("lpd")      # 1/(pwseg+eps)
            V.tensor_scalar(lpd[:], pwseg, EPS, None, OP.add)
            V.reciprocal(lpd[:], lpd[:])

            la = rtile("la")        # qa*attseg/(count+eps) * has
            V.tensor_tensor(la[:], attseg, qaF[:], OP.mult)
            V.tensor_tensor(la[:], la[:], rc[:], OP.mult)
            V.tensor_tensor(la[:], la[:], has[:], OP.mult)

            # rep_k = (repF - qseg + qminseg)*qa*rnc*has   (relu-form)
            lr = rtile("lr")
            V.tensor_tensor(lr[:], repF[:], qminseg, OP.add)
            V.tensor_tensor(lr[:], lr[:], qseg, OP.subtract)
            V.tensor_tensor(lr[:], lr[:], qaF[:], OP.mult)
            V.tensor_tensor(lr[:], lr[:], rnc[:], OP.mult)
            V.tensor_tensor(lr[:], lr[:], has[:], OP.mult)

            lb = rtile("lb")        # has*(1 - beta_alpha) = has*(2 - Bglob)
            V.tensor_scalar(lb[:], BglobF[:], -1.0, 2.0, OP.mult, OP.add)
            V.tensor_tensor(lb[:], lb[:], has[:], OP.mult)

            lp = rtile("lp")        # has*paynum/(payden+eps)
            V.tensor_tensor(lp[:], lpd[:], payseg, OP.mult)
            V.tensor_tensor(lp[:], lp[:], has[:], OP.mult)

            lsum = rtile("lsum")
            V.tensor_tensor(lsum[:], la[:], lr[:], OP.add)
            V.tensor_tensor(lsum[:], lsum[:], lb[:], OP.add)
            V.tensor_tensor(lsum[:], lsum[:], lp[:], OP.add)
            fin = io.tile([1, 2], f32, name="fin")
            V.tensor_reduce(fin[0:1, 0:1], lsum[:], mybir.AxisListType.X, OP.add)
            V.tensor_reduce(fin[0:1, 1:2], has[:], mybir.AxisListType.X, OP.add)

            # total = lsum/n_obj + nb/(nn+eps) + 0.001*xsq/(2N)
            nobj = io.tile([1, 1], f32, name="nobj")
            V.tensor_scalar(nobj[:], fin[0:1, 1:2], EPS, None, OP.add)
            V.reciprocal(nobj[:], nobj[:])
            tot = io.tile([1, 1], f32, name="tot")
            V.tensor_tensor(tot[:], fin[0:1, 0:1], nobj[:], OP.mult)
            nden = io.tile([1, 1], f32, name="nden")
            V.tensor_scalar(nden[:], extras_g[0:1, 1:2], EPS, None, OP.add)
            V.reciprocal(nden[:], nden[:])
            V.tensor_tensor(nden[:], nden[:], extras_g[0:1, 0:1], OP.mult)
            V.tensor_tensor(tot[:], tot[:], nden[:], OP.add)
            lcc = io.tile([1, 1], f32, name="lcc")
            SC.activation(lcc[:], extras_g[0:1, 2:3], AF.Copy,
                          scale=0.001 / (2.0 * N))
            V.tensor_tensor(tot[:], tot[:], lcc[:], OP.add)
            nc.sync.dma_start(out_d.ap(), tot[:])

    nc.compile()
    return nc


def _host_prep(inputs):
    """Slice, pad and re-layout the full inputs into 8 per-core input maps."""
    def lay(a2):                       # [SP, w] -> [128, T, w]
        w = a2.shape[1]
        r = a2.reshape(T, P, w).transpose(1, 0, 2)
        return np.ascontiguousarray(r.astype(np.float32))

    in_maps = []
    for c in range(NCORES):
        sl = slice(c * S, (c + 1) * S)

        def pad(a, fill=0.0):
            out = np.full((SP, a.shape[1]), fill, np.float32)
            out[:S] = a[sl]
            return out

        tidx = np.full((SP, 1), -2.0, np.float32)
        tidx[:S, 0] = inputs["t_idx"][sl, 0].astype(np.float32)
        valid = np.zeros((SP, 1), np.float32)
        valid[:S] = 1.0
        m = {
            "beta_r": lay(pad(inputs["pred_beta"]))[:, :, 0],
            "cc": lay(pad(inputs["pred_ccoords"])),
            "pE": lay(pad(inputs["pred_energy"]))[:, :, 0],
            "ppos": lay(pad(inputs["pred_pos"])),
            "ptime": lay(pad(inputs["pred_time"]))[:, :, 0],
            "pid": lay(pad(inputs["pred_id"])),
            "tE": lay(pad(inputs["t_energy"]))[:, :, 0],
            "tpos": lay(pad(inputs["t_pos"])),
            "ttime": lay(pad(inputs["t_time"]))[:, :, 0],
            "tidx": lay(tidx)[:, :, 0],
            "valid": lay(valid)[:, :, 0],
        }
        m = {k: np.ascontiguousarray(v) for k, v in m.items()}
        in_maps.append(m)
    return in_maps


def _run(inputs, trace=False):
    from concourse import bass_utils
    if "nc" not in _CACHE:
        _CACHE["nc"] = _build()
    nc = _CACHE["nc"]
    in_maps = _host_prep(inputs)
    res = bass_utils.run_bass_kernel_spmd(
        nc, in_maps, core_ids=list(range(NCORES)), trace=trace)
    return res


def kernel(**inputs):
    res = _run(inputs, trace=False)
    val = np.float32(res.results[0]["out"][0, 0])
    return np.array(val, dtype=np.float32)[()]


if __name__ == "__main__":
    d = np.load("/tmp/inputs.npz")
    inp = {k: d[k] for k in d.files}
    print("kernel:", kernel(**inp))


# revision 33
# speedup vs baseline: 1.2116x; 1.0658x over previous
"""Trainium2 Bass kernel for LLFullObjectCondensation loss (N=80000, K=512, C=2).

Strategy (8 NeuronCores, data-parallel over hits):
  - Each core gets a 10000-hit shard (padded to 79*128=10112), laid out [128, 79].
  - P1: per-hit quantities (q, payload, weights) as full-width [128,79] ops.
  - P2: per-object max of (beta+1)-weighted one-hot tiles (DVE build + running
        max), 4 PE transposes + reductions -> Bloc, packed to row BlocF [1,K].
  - AllReduce-max of BlocF; P3 selection pass overlaps: Isel = (bm == BlocB),
        PE matmul with sel3 stationary -> selPT [3,K] rows, gated by
        keepF = (BlocF == BglobF), AllReduce-add -> global x_alpha/q_alpha rows.
  - P5 loop 1 (software-pipelined): d2 via PE matmul (contract-4 trick), sqrt
        on ACT, min on DVE, repulsion row-sums via PE, self-distance gather via
        DVE row-reduce with the weighted one-hot.
  - P5 loop 2: segment sums as PE matmuls, rhs_seg [128,6] stationary, bm
        moving -> segPT [6,K] rows.
  - AllReduce-add of all per-object rows, then row-layout [1,K] assembly of
        the scalar loss.
All one-hot builds and elementwise work run on DVE/ACT (GPSIMD's software
tensor_scalar measured ~8us per [128,512] tile vs DVE's ~0.55us).
"""
import sys
import numpy as np

for _p in ("/opt/trn_rl_repo", "/root/.axon_site/_ro/trn_rl_repo"):
    if _p not in sys.path:
        sys.path.append(_p)

N = 80000
K = 512
NCORES = 8
S = N // NCORES          # 10000 hits per core
P = 128
T = 79                   # tiles per core, T*P = 10112 >= S
SP = T * P
KB = K // P              # 4 k-blocks
EPS = 1e-9
SQ_BIAS = 2e-2           # reference uses 1e-6; extra margin absorbs fp32r
                         # matmul rounding of the expanded |x|^2-2x.a+|a|^2
                         # form so sqrt never sees a negative input (measured:
                         # 1e-3 still went NaN -> fp32r product error ~5e-3 abs
                         # on O(20) terms; the 0.02 bias shifts the hinge by
                         # ~1.2e-3 relative on the total loss, gate is 2e-2)

_CACHE = {}


def _build(cc_mode='all'):
    import concourse.bass as bass
    import concourse.bacc as bacc
    import concourse.mybir as mybir
    import concourse.tile as tile
    from concourse import masks

    f32 = mybir.dt.float32
    f32r = mybir.dt.float32r
    i32 = mybir.dt.int32
    u16 = mybir.dt.uint16
    f16 = mybir.dt.float16
    AF = mybir.ActivationFunctionType
    OP = mybir.AluOpType

    nc = bacc.Bacc("TRN2", target_bir_lowering=False, debug=False,
                   num_devices=NCORES)

    di = {}
    def din(name, shape):
        di[name] = nc.dram_tensor(name, shape, f32, kind="ExternalInput")
        return di[name]

    din("beta_r", [P, T])
    din("cc", [P, T, 2])
    din("pE", [P, T])
    din("ppos", [P, T, 2])
    din("ptime", [P, T])
    din("pid", [P, T, 6])
    din("tE", [P, T])
    din("tpos", [P, T, 2])
    din("ttime", [P, T])
    din("tidx", [P, T])
    din("valid", [P, T])
    out_d = nc.dram_tensor("out", [1, 1], f32, kind="ExternalOutput")

    with tile.TileContext(nc) as tc:
        with (
            tc.tile_pool(name="const", bufs=1) as cpool,
            tc.tile_pool(name="io", bufs=1) as io,
            tc.tile_pool(name="dram", bufs=1, space="DRAM") as dram,
            tc.tile_pool(name="psA", bufs=2, space="PSUM") as psA,
            tc.tile_pool(name="acc", bufs=1, space="PSUM") as accp,
        ):
            # ---------- constants ----------
            ident = cpool.tile([P, P], f32)
            masks.make_identity(nc, ident[:])
            iotaI = cpool.tile([P, K], i32)
            nc.gpsimd.iota(iotaI[:], pattern=[[1, K]], base=0,
                           channel_multiplier=0)
            iotaF = cpool.tile([P, K], f32)
            nc.vector.tensor_copy(iotaF[:], iotaI[:])
            iotaH = cpool.tile([P, K], f16)
            nc.vector.tensor_copy(iotaH[:], iotaI[:])
            onescol = cpool.tile([P, 1], f32)
            nc.vector.memset(onescol[:], 1.0)
            onesrow = cpool.tile([1, P], f32)
            nc.vector.memset(onesrow[:], 1.0)

            _cb = {}
            def cbias(val):
                """[128,1] constant column for activation bias operands."""
                if val not in _cb:
                    ct = cpool.tile([P, 1], f32, name=f"cb{len(_cb)}")
                    nc.vector.memset(ct[:], val)
                    _cb[val] = ct
                return _cb[val][:]

            # ---------- load inputs ----------
            sb = {}
            for name, h in di.items():
                t_sb = io.tile(list(h.shape), f32, name=f"sb_{name}")
                nc.sync.dma_start(t_sb[:], h.ap())
                sb[name] = t_sb

            # ---------- P1: per-hit prep (all [128,T]-wide ops) ----------
            V = nc.vector
            SC = nc.scalar

            def wtile(name, shape=None, dtype=None):
                return io.tile(shape or [P, T], dtype or f32, name=name)
            u8 = mybir.dt.uint8

            beta = wtile("beta")
            V.tensor_scalar(beta[:], sb["beta_r"][:], 1e-6, 1.0 - 1e-6,
                            OP.max, OP.min)
            betap1 = wtile("betap1")
            SC.activation(betap1[:], beta[:], AF.Identity, bias=cbias(1.0))
            rb1 = wtile("rb1")
            V.reciprocal(rb1[:], betap1[:])
            onem = wtile("onem")
            SC.activation(onem[:], beta[:], AF.Identity, bias=cbias(1.0), scale=-1.0)
            recm = wtile("recm")
            V.reciprocal(recm[:], onem[:])
            ratio = wtile("ratio")
            V.tensor_tensor(ratio[:], betap1[:], recm[:], OP.mult)

            ntidx = wtile("ntidx")           # -t_idx (ACT bias operand)
            V.tensor_scalar(ntidx[:], sb["tidx"][:], -1.0, None, OP.mult)
            nbetap1 = wtile("nbetap1")       # -(beta+1) (ACT scale operand)
            SC.activation(nbetap1[:], beta[:], AF.Identity, bias=cbias(-1.0),
                          scale=-1.0)
            is_noise = wtile("is_noise")
            V.tensor_scalar(is_noise[:], sb["tidx"][:], -1.0, None, OP.is_equal)
            is_obj = wtile("is_obj")
            V.tensor_scalar(is_obj[:], sb["tidx"][:], 0.0, None, OP.is_ge)

            # energy weights w = relu(min(wr,1)) ; wr=(tE-0.5)/9.5
            wr = wtile("wr")
            SC.activation(wr[:], sb["tE"][:], AF.Identity, bias=cbias(-0.5 / 9.5),
                          scale=1.0 / 9.5)
            ew = wtile("ew")
            V.tensor_scalar(ew[:], wr[:], 1.0, 0.0, OP.min, OP.max)
            pw = wtile("pw")
            V.tensor_tensor(pw[:], beta[:], ew[:], OP.mult)
            V.tensor_tensor(pw[:], pw[:], is_obj[:], OP.mult)

            # --- energy loss pieces (pre-transcendental) ---
            ediff_r = wtile("ediff_r")
            V.tensor_tensor(ediff_r[:], sb["tE"][:], sb["pE"][:], OP.subtract)
            ediff = wtile("ediff")
            SC.activation(ediff[:], ediff_r[:], AF.Abs)
            ed2 = wtile("ed2")
            V.tensor_tensor(ed2[:], ediff[:], ediff[:], OP.mult)
            ed001 = wtile("ed001")
            SC.activation(ed001[:], ediff[:], AF.Copy, scale=0.001)

            # --- position loss pieces ---
            dpos = wtile("dpos", [P, T, 2])
            V.tensor_tensor(dpos[:], sb["tpos"][:], sb["ppos"][:], OP.subtract)
            V.tensor_tensor(dpos[:], dpos[:], dpos[:], OP.mult)
            d2p = wtile("d2p")
            V.tensor_tensor(d2p[:], dpos[:, :, 0], dpos[:, :, 1], OP.add)

            # --- timing loss pieces ---
            dtm = wtile("dtm")
            V.tensor_tensor(dtm[:], sb["ttime"][:], sb["ptime"][:], OP.subtract)
            adt = wtile("adt")
            SC.activation(adt[:], dtm[:], AF.Abs)
            dt2 = wtile("dt2")
            V.tensor_tensor(dt2[:], dtm[:], dtm[:], OP.mult)
            lint = wtile("lint")
            SC.activation(lint[:], adt[:], AF.Identity, bias=cbias(-4.0), scale=4.0)
            ltt = wtile("ltt", dtype=u8)
            V.tensor_scalar(ltt[:], adt[:], 2.0, None, OP.is_lt)
            ht = wtile("ht")
            V.select(ht[:], ltt[:], dt2[:], lint[:])
            yt = wtile("yt")
            SC.activation(yt[:], ht[:], AF.Copy, scale=1.0 / 6.0)

            # --- classification loss ---
            pid2 = wtile("pid2", [P, T, 6])
            V.tensor_tensor(pid2[:], sb["pid"][:], sb["pid"][:], OP.mult)
            cred = wtile("cred")
            V.tensor_reduce(cred[:], pid2[:], mybir.AxisListType.X, OP.add)

            # --- transcendental block: Exp, then Sqrt, then Ln (grouped to
            # limit ACT table swaps) ---
            ex = wtile("ex")
            SC.activation(ex[:], ed2[:], AF.Exp, scale=-0.1)
            xp = wtile("xp")
            SC.activation(xp[:], d2p[:], AF.Sqrt, bias=cbias(0.01), scale=0.01)

            lnr = wtile("lnr")
            SC.activation(lnr[:], ratio[:], AF.Ln)
            # q = (0.5*ln(ratio))^2 + 0.1, zeroed on padding
            halfln = wtile("halfln")
            SC.activation(halfln[:], lnr[:], AF.Copy, scale=0.5)
            q = wtile("q")
            V.tensor_tensor(q[:], halfln[:], halfln[:], OP.mult)
            V.scalar_tensor_tensor(q[:], q[:], 0.1, sb["valid"][:],
                                   OP.add, OP.mult)

            # energy softclip
            ye = wtile("ye")
            V.tensor_tensor(ye[:], ex[:], ed001[:], OP.add)
            lnye = wtile("lnye")
            SC.activation(lnye[:], ye[:], AF.Ln, bias=cbias(1.0))
            gte = wtile("gte", dtype=u8)
            V.tensor_scalar(gte[:], ye[:], 1.0, None, OP.is_gt)
            esc = wtile("esc")
            V.select(esc[:], gte[:], lnye[:], ye[:])

            # position huber + softclip
            xp2 = wtile("xp2")
            V.tensor_tensor(xp2[:], xp[:], xp[:], OP.mult)
            linp = wtile("linp")
            SC.activation(linp[:], xp[:], AF.Identity, bias=cbias(-100.0), scale=20.0)
            ltp = wtile("ltp", dtype=u8)
            V.tensor_scalar(ltp[:], xp[:], 10.0, None, OP.is_lt)
            hp = wtile("hp")
            V.select(hp[:], ltp[:], xp2[:], linp[:])
            yp = wtile("yp")
            SC.activation(yp[:], hp[:], AF.Copy, scale=1.0 / 3.0)
            lnyp = wtile("lnyp")
            SC.activation(lnyp[:], yp[:], AF.Ln, bias=cbias(1.0))
            gtp = wtile("gtp", dtype=u8)
            V.tensor_scalar(gtp[:], yp[:], 1.0, None, OP.is_gt)
            psc = wtile("psc")
            V.select(psc[:], gtp[:], lnyp[:], yp[:])

            # timing softclip
            lnyt = wtile("lnyt")
            SC.activation(lnyt[:], yt[:], AF.Ln, bias=cbias(1.0))
            gtt = wtile("gtt", dtype=u8)
            V.tensor_scalar(gtt[:], yt[:], 1.0, None, OP.is_gt)
            tsc = wtile("tsc")
            V.select(tsc[:], gtt[:], lnyt[:], yt[:])

            # payload = 10*esc + 3*psc + 6*tsc + (1e-8/6)*cred
            esc10 = wtile("esc10")
            SC.activation(esc10[:], esc[:], AF.Copy, scale=10.0)
            pay = wtile("pay")
            V.scalar_tensor_tensor(pay[:], psc[:], 3.0, esc10[:],
                                   OP.mult, OP.add)
            V.scalar_tensor_tensor(pay[:], tsc[:], 6.0, pay[:],
                                   OP.mult, OP.add)
            V.scalar_tensor_tensor(pay[:], cred[:], 1e-8 / 6.0, pay[:],
                                   OP.mult, OP.add)
            paypw = wtile("paypw")
            V.tensor_tensor(paypw[:], pay[:], pw[:], OP.mult)

            # selection rhs: [x0, x1, q] (stationary operand for P3 matmuls)
            sel3 = wtile("sel3", [P, T, 3], dtype=f32r)
            SC.activation(sel3[:, :, 0:2], sb["cc"][:], AF.Copy)
            V.tensor_copy(sel3[:, :, 2], q[:])

            # d2-matmul lhsT quantities [-2x0, -2x1, 1, |x|^2] packed [P,T,4]
            prep4 = wtile("prep4", [P, T, 4])
            SC.activation(prep4[:, :, 0:2], sb["cc"][:], AF.Copy, scale=-2.0)
            V.memset(prep4[:, :, 2], 1.0)
            ccsq = wtile("ccsq", [P, T, 2])
            V.tensor_tensor(ccsq[:], sb["cc"][:], sb["cc"][:], OP.mult)
            V.tensor_tensor(prep4[:, :, 3], ccsq[:, :, 0], ccsq[:, :, 1],
                            OP.add)

            # extras: [noise*beta, noise, |x|^2, q] free-reduced to [P,4],
            # then partition-reduced to a [1,4] row via PE (ready for AR3)
            extras = io.tile([P, 4], f32, name="extras")
            nb_t = wtile("nb_t")
            V.tensor_tensor(nb_t[:], is_noise[:], beta[:], OP.mult)
            V.tensor_reduce(extras[:, 0:1], nb_t[:], mybir.AxisListType.X, OP.add)
            V.tensor_reduce(extras[:, 1:2], is_noise[:], mybir.AxisListType.X, OP.add)
            V.tensor_reduce(extras[:, 2:3], prep4[:, :, 3], mybir.AxisListType.X, OP.add)
            V.tensor_reduce(extras[:, 3:4], q[:], mybir.AxisListType.X, OP.add)
            extrasF = io.tile([1, 4], f32, name="extrasF")
            with tc.tile_pool(name="exp", bufs=1, space="PSUM") as exp_p:
                exPS = exp_p.tile([1, 4], f32, name="exPS")
                nc.tensor.matmul(exPS[:], onescol[:], extras[:],
                                 start=True, stop=True)
                SC.activation(extrasF[:], exPS[:], AF.Copy)

            # transpose prep4 -> lhsT4r [4, T, 128] (f32r, rounded at the
            # ACT evacuation so the fp32r d2 matmul accepts it)
            lhsT4r = io.tile([4, T, P], f32r, name="lhsT4r")
            for r in range(4):
                tp = psA.tile([P, P], f32, name="tpose4", tag="tpose")
                nc.tensor.transpose(tp[0:T, :], prep4[:, :, r], ident[:])
                stage = io.tile([T, P], f32r, name=f"tstage{r}")
                SC.activation(stage[:], tp[0:T, :], AF.Copy)
                nc.sync.dma_start(lhsT4r[r:r + 1, :, :], stage[:])

            # ---------- P2: local per-object max of (beta+1)-weighted one-hot
            # (DVE builds; bm[p,k] = (iota[k]==tidx[p,t]) * (beta[p,t]+1)) ----
            runmax = io.tile([P, K], f32, name="runmax")
            V.memset(runmax[:], 0.0)
            with tc.tile_pool(name="bmp", bufs=4) as bmp:
                for t in range(T):
                    bm = bmp.tile([P, K], f32, name="bm")
                    if t % 2 == 0:
                        V.tensor_scalar(bm[:], iotaF[:],
                                        sb["tidx"][:, t:t + 1],
                                        betap1[:, t:t + 1],
                                        OP.is_equal, OP.mult)
                    else:
                        # exact one-hot on ACT: relu(b1 - b1*|iota - tidx|)
                        ab = bmp.tile([P, K], f32, name="ab")
                        SC.activation(ab[:], iotaF[:], AF.Abs,
                                      bias=ntidx[:, t:t + 1])
                        SC.activation(bm[:], ab[:], AF.Relu,
                                      scale=nbetap1[:, t:t + 1],
                                      bias=betap1[:, t:t + 1])
                    V.tensor_tensor(runmax[:], runmax[:], bm[:], OP.max)

            # partition-reduce runmax -> Bloc [128,4] (k = 128*b + p)
            Bloc = io.tile([P, KB], f32, name="Bloc")
            for b in range(KB):
                tp = psA.tile([P, P], f32, name="tpose", tag="tpose")
                nc.tensor.transpose(tp[:], runmax[:, b * P:(b + 1) * P], ident[:])
                V.reduce_max(Bloc[:, b:b + 1], tp[:], axis=mybir.AxisListType.X)

            # row layout: BlocF[0, 128*b+p] = Bloc[p, b]
            BlocF = io.tile([1, K], f32, name="BlocF")
            for b in range(KB):
                nc.sync.dma_start(BlocF[0:1, b * P:(b + 1) * P], Bloc[:, b:b + 1])

            # ---------- P4a: AllReduce-max of BlocF (overlaps with P3) -------
            arm_in = dram.tile([1, K], f32, name="arm_in")
            arm_out = dram.tile([1, K], f32, name="arm_out", addr_space="Shared")
            nc.sync.dma_start(arm_in[0:1, :], BlocF[:])
            if cc_mode in ('all', 'first', 'two'):
                nc.gpsimd.collective_compute(
                    "AllReduce", OP.max,
                    replica_groups=[list(range(NCORES))],
                    ins=[arm_in[:]], outs=[arm_out[:]],
                )
            else:
                nc.sync.dma_start(arm_out[:], arm_in[:])
            BglobF = io.tile([1, K], f32, name="BglobF")
            nc.sync.dma_start(BglobF[:], arm_out[0:1, :])

            # broadcast BlocF across partitions via PE: ones[1,P].T @ BlocF
            BlocB = io.tile([P, K], f32, name="BlocB")
            with tc.tile_pool(name="bcp", bufs=1, space="PSUM") as bcp:
                blocps = bcp.tile([P, K], f32, name="blocps")
                nc.tensor.matmul(blocps[:], onesrow[:], BlocF[:],
                                 start=True, stop=True)
                SC.activation(BlocB[:], blocps[:], AF.Copy)

            # ---------- P3: selection segment-sums -> selPT rows [3, K] ------
            with (
                tc.tile_pool(name="selpp", bufs=1, space="PSUM") as selpp,
                tc.tile_pool(name="bmp3", bufs=3) as bmp3,
            ):
                selPT = selpp.tile([3, K], f32, name="selPT")
                V.memset(selPT[:], 0.0)
                for t in range(T):
                    bm = bmp3.tile([P, K], f32, name="bm3")
                    if t % 2 == 0:
                        V.tensor_scalar(bm[:], iotaF[:],
                                        sb["tidx"][:, t:t + 1],
                                        betap1[:, t:t + 1],
                                        OP.is_equal, OP.mult)
                    else:
                        ab = bmp3.tile([P, K], f32, name="ab3")
                        SC.activation(ab[:], iotaF[:], AF.Abs,
                                      bias=ntidx[:, t:t + 1])
                        SC.activation(bm[:], ab[:], AF.Relu,
                                      scale=nbetap1[:, t:t + 1],
                                      bias=betap1[:, t:t + 1])
                    Isel = bmp3.tile([P, K], f32r, name="Isel")
                    V.tensor_tensor(Isel[:], bm[:], BlocB[:], OP.is_equal)
                    nc.tensor.matmul(selPT[:], sel3[:, t, :], Isel[:],
                                     start=False, stop=(t == T - 1),
                                     skip_group_check=True)
                selsbT = io.tile([3, K], f32, name="selsbT")
                SC.activation(selsbT[:], selPT[:], AF.Copy)

            # gate by global-winner mask and AllReduce-add.  Compute engines
            # must start at partition 0/32/64/96, so replicate keepF to 3
            # partitions via DMA and gate with one [3,K] multiply.
            keepF = io.tile([1, K], f32, name="keepF")
            V.tensor_tensor(keepF[:], BlocF[:], BglobF[:], OP.is_equal)
            keep3 = io.tile([3, K], f32, name="keep3")
            for r in range(3):
                nc.sync.dma_start(keep3[r:r + 1, :], keepF[:])
            sel_cT = io.tile([3, K], f32, name="sel_cT")
            V.tensor_tensor(sel_cT[:], selsbT[:], keep3[:], OP.mult)
            ar2_in = dram.tile([1, 3 * K], f32, name="ar2_in")
            ar2_out = dram.tile([1, 3 * K], f32, name="ar2_out",
                                addr_space="Shared")
            nc.sync.dma_start(ar2_in[0:1, :], sel_cT[:])   # row-major pack
            if cc_mode in ('all', 'two'):
                nc.gpsimd.collective_compute(
                    "AllReduce", OP.add,
                    replica_groups=[list(range(NCORES))],
                    ins=[ar2_in[:]], outs=[ar2_out[:]],
                )
            else:
                nc.sync.dma_start(ar2_out[:], ar2_in[:])

            # rhsD2 rows: [xa0; xa1; |xa|^2; 1].  |xa|^2 is computed on
            # partition 0 (xa0F/xa1F row tiles) and DMA'd into row 2.
            rhsD2 = io.tile([4, K], f32, name="rhsD2")
            V.memset(rhsD2[:], 1.0)
            nc.sync.dma_start(
                rhsD2[0:2, :],
                ar2_out[0:1, 0:2 * K].rearrange("o (r k) -> (o r) k", r=2))
            xa0F = io.tile([1, K], f32, name="xa0F")
            nc.sync.dma_start(xa0F[:], ar2_out[0:1, 0:K])
            xa1F = io.tile([1, K], f32, name="xa1F")
            nc.sync.dma_start(xa1F[:], ar2_out[0:1, K:2 * K])
            qaF = io.tile([1, K], f32, name="qaF")
            nc.sync.dma_start(qaF[:], ar2_out[0:1, 2 * K:3 * K])
            xsqF = io.tile([1, K], f32, name="xsqF")
            xsq_t = io.tile([1, K], f32, name="xsq_t")
            V.tensor_tensor(xsq_t[:], xa1F[:], xa1F[:], OP.mult)
            V.tensor_tensor(xsqF[:], xa0F[:], xa0F[:], OP.mult)
            V.tensor_tensor(xsqF[:], xsqF[:], xsq_t[:], OP.add)
            nc.sync.dma_start(rhsD2[2:3, :], xsqF[:])
            rhsD2r = io.tile([4, K], f32r, name="rhsD2r")
            V.tensor_copy(rhsD2r[:], rhsD2[:])

            # prebuild the first NPRE segment one-hots around the AR2 window
            NPRE = 36
            bm6pre = io.tile([P, NPRE, K], f16, name="bm6pre")
            for t in range(NPRE):
                V.tensor_scalar(bm6pre[:, t, :], iotaH[:],
                                sb["tidx"][:, t:t + 1],
                                betap1[:, t:t + 1], OP.is_equal, OP.mult)

            qr = wtile("qr", dtype=f32r)      # rounded copy for fp32r matmul
            V.tensor_copy(qr[:], q[:])

            # ---------- P5 loop 1: d2 block, rep row-sums, self-distance -----
            # software-pipelined by one stage: d2 matmul for t+1 issues before
            # the rep matmul for t so the PE never blocks behind the sqrt.
            # rep accumulates q * relu(1 - s) directly (hinge on ACT).
            gstD = io.tile([P, T], f32, name="gstD")   # (beta+1)*s_self
            repP = accp.tile([1, K], f32, name="repP")
            V.memset(repP[:], 0.0)
            scr = io.tile([P, K], f16, name="scr")         # ttr full-out scratch
            with (
                tc.tile_pool(name="d2pool", bufs=3, space="PSUM") as d2pool,
                tc.tile_pool(name="sp", bufs=3) as sp,
                tc.tile_pool(name="bmp5", bufs=3) as bmp5,
            ):
                d2tiles = {}
                def d2mm(t):
                    d2ps = d2pool.tile([P, K], f32, name="d2ps")
                    nc.tensor.matmul(d2ps[:], lhsT4r[0:4, t, :], rhsD2r[:],
                                     start=True, stop=True)
                    d2tiles[t] = d2ps
                d2mm(0)
                sSp = None
                for t in range(T):
                    if t + 1 < T:
                        d2mm(t + 1)
                    d2ps = d2tiles.pop(t)
                    bm = bmp5.tile([P, K], f16, name="bm5")
                    V.tensor_scalar(bm[:], iotaH[:], sb["tidx"][:, t:t + 1],
                                    betap1[:, t:t + 1], OP.is_equal, OP.mult)
                    half = t % 2
                    if half == 0:
                        sSp = sp.tile([P, 2, K], f16, name="sSp")
                        rlup = sp.tile([P, 2, K], f32r, name="rlup")
                    sS = sSp[:, half, :]
                    SC.activation(sS, d2ps[:], AF.Sqrt, bias=cbias(SQ_BIAS))
                    # hinge for the pair in one ACT pass (halves the per-op
                    # overhead); last odd tile runs as a single
                    if half == 1:
                        SC.activation(rlup[:], sSp[:], AF.Relu,
                                      bias=cbias(1.0), scale=-1.0)
                        for h in (0, 1):
                            nc.tensor.matmul(repP[:], qr[:, t - 1 + h:t + h],
                                             rlup[:, h, :],
                                             start=False, stop=False,
                                             skip_group_check=True)
                    elif t == T - 1:
                        SC.activation(rlup[:, 0, :], sS, AF.Relu,
                                      bias=cbias(1.0), scale=-1.0)
                        nc.tensor.matmul(repP[:], qr[:, t:t + 1],
                                         rlup[:, 0, :],
                                         start=False, stop=True,
                                         skip_group_check=True)
                    V.scalar_tensor_tensor(
                        scr[:], bm[:], 1.0, sS, OP.bypass, OP.mult,
                        accum_out=gstD[:, t:t + 1])

            # ---------- global per-hit math for segment rhs ----------
            qrb = wtile("qrb")
            V.tensor_tensor(qrb[:], q[:], rb1[:], OP.mult)
            sself = wtile("sself")              # sqrt(d2_self + SQ_BIAS)
            V.tensor_tensor(sself[:], gstD[:], rb1[:], OP.mult)
            G2 = wtile("G2")                    # d2_self
            V.tensor_tensor(G2[:], sself[:], sself[:], OP.mult)
            V.tensor_scalar(G2[:], G2[:], SQ_BIAS, None, OP.subtract)
            s2 = wtile("s2")                    # min(s_self, 1)
            V.tensor_scalar(s2[:], sself[:], 1.0, None, OP.min)
            rhs_seg = io.tile([P, T, 6], f16, name="rhs_seg")
            # att' = q*d2_self/(b+1)
            V.tensor_tensor(rhs_seg[:, :, 0], G2[:], qrb[:], OP.mult)
            # qmin' = q*min(s_self,1)/(b+1)
            V.tensor_tensor(rhs_seg[:, :, 1], s2[:], qrb[:], OP.mult)
            V.tensor_tensor(rhs_seg[:, :, 2], sb["valid"][:], rb1[:], OP.mult)
            V.tensor_tensor(rhs_seg[:, :, 3], pw[:], rb1[:], OP.mult)
            V.tensor_tensor(rhs_seg[:, :, 4], paypw[:], rb1[:], OP.mult)
            V.tensor_copy(rhs_seg[:, :, 5], qrb[:])

            # ---------- P5 loop 2: segment sums -> segPT rows [6, K] ---------
            segPT = accp.tile([6, K], f32, name="segPT")
            V.memset(segPT[:], 0.0)
            with tc.tile_pool(name="bmp6", bufs=3) as bmp6:
                for t in range(T):
                    if t < NPRE:
                        bmap = bm6pre[:, t, :]
                    else:
                        bm = bmp6.tile([P, K], f16, name="bm6")
                        V.tensor_scalar(bm[:], iotaH[:],
                                        sb["tidx"][:, t:t + 1],
                                        betap1[:, t:t + 1],
                                        OP.is_equal, OP.mult)
                        bmap = bm[:]
                    nc.tensor.matmul(segPT[:], rhs_seg[:, t, :], bmap,
                                     start=False, stop=(t == T - 1),
                                     skip_group_check=True)

            # ---------- P6: AllReduce of per-object rows ----------
            segsbT = io.tile([6, K], f32, name="segsbT")
            SC.activation(segsbT[:], segPT[:], AF.Copy)
            repsb = io.tile([1, K], f32, name="repsb")
            SC.activation(repsb[:], repP[:], AF.Copy)

            NSEG = 6 * K
            NTOT = NSEG + K + 4
            ar_in = dram.tile([1, NTOT], f32, name="ar_in")
            ar_out = dram.tile([1, NTOT], f32, name="ar_out", addr_space="Shared")
            nc.sync.dma_start(ar_in[0:1, 0:NSEG], segsbT[:])     # row-major
            nc.sync.dma_start(ar_in[0:1, NSEG:NSEG + K], repsb[:])
            nc.sync.dma_start(ar_in[0:1, NSEG + K:NTOT], extrasF[:])
            if cc_mode == 'all':
                nc.gpsimd.collective_compute(
                    "AllReduce", OP.add,
                    replica_groups=[list(range(NCORES))],
                    ins=[ar_in[:]], outs=[ar_out[:]],
                )
            else:
                nc.sync.dma_start(ar_out[:], ar_in[:])
            # unpack per-object rows (partition-0 tiles; compute engines
            # cannot start at partitions 1..5)
            seg_rows = []
            for r in range(6):
                rt = io.tile([1, K], f32, name=f"segrow{r}")
                nc.sync.dma_start(rt[:], ar_out[0:1, r * K:(r + 1) * K])
                seg_rows.append(rt)
            repF = io.tile([1, K], f32, name="repF")
            nc.sync.dma_start(repF[:], ar_out[0:1, NSEG:NSEG + K])
            extras_g = io.tile([1, 4], f32, name="extras_g")
            nc.sync.dma_start(extras_g[:], ar_out[0:1, NSEG + K:NTOT])

            # ---------- P7: assembly ([1,K] rows) ----------
            attseg = seg_rows[0][:]
            qminseg = seg_rows[1][:]
            count = seg_rows[2][:]
            pwseg = seg_rows[3][:]
            payseg = seg_rows[4][:]
            qseg = seg_rows[5][:]

            def rtile(name):
                return io.tile([1, K], f32, name=name)

            has = rtile("has")
            V.tensor_scalar(has[:], count, 0.0, None, OP.is_gt)
            rc = rtile("rc")        # 1/(count+eps)
            V.tensor_scalar(rc[:], count, EPS, None, OP.add)
            V.reciprocal(rc[:], rc[:])
            rnc = rtile("rnc")      # 1/(N-count+eps)
            V.tensor_scalar(rnc[:], count, -1.0, float(N) + EPS,
                            OP.mult, OP.add)
            V.reciprocal(rnc[:], rnc[:])
            lpd = rtile("lpd")      # 1/(pwseg+eps)
            V.tensor_scalar(lpd[:], pwseg, EPS, None, OP.add)
            V.reciprocal(lpd[:], lpd[:])

            la = rtile("la")        # qa*attseg/(count+eps) * has
            V.tensor_tensor(la[:], attseg, qaF[:], OP.mult)
            V.tensor_tensor(la[:], la[:], rc[:], OP.mult)
            V.tensor_tensor(la[:], la[:], has[:], OP.mult)

            # rep_k = (repF - qseg + qminseg)*qa*rnc*has   (relu-form)
            lr = rtile("lr")
            V.tensor_tensor(lr[:], repF[:], qminseg, OP.add)
            V.tensor_tensor(lr[:], lr[:], qseg, OP.subtract)
            V.tensor_tensor(lr[:], lr[:], qaF[:], OP.mult)
            V.tensor_tensor(lr[:], lr[:], rnc[:], OP.mult)
            V.tensor_tensor(lr[:], lr[:], has[:], OP.mult)

            lb = rtile("lb")        # has*(1 - beta_alpha) = has*(2 - Bglob)
            V.tensor_scalar(lb[:], BglobF[:], -1.0, 2.0, OP.mult, OP.add)
            V.tensor_tensor(lb[:], lb[:], has[:], OP.mult)

            lp = rtile("lp")        # has*paynum/(payden+eps)
            V.tensor_tensor(lp[:], lpd[:], payseg, OP.mult)
            V.tensor_tensor(lp[:], lp[:], has[:], OP.mult)

            lsum = rtile("lsum")
            V.tensor_tensor(lsum[:], la[:], lr[:], OP.add)
            V.tensor_tensor(lsum[:], lsum[:], lb[:], OP.add)
            V.tensor_tensor(lsum[:], lsum[:], lp[:], OP.add)
            fin = io.tile([1, 2], f32, name="fin")
            V.tensor_reduce(fin[0:1, 0:1], lsum[:], mybir.AxisListType.X, OP.add)
            V.tensor_reduce(fin[0:1, 1:2], has[:], mybir.AxisListType.X, OP.add)

            # total = lsum/n_obj + nb/(nn+eps) + 0.001*xsq/(2N)
            nobj = io.tile([1, 1], f32, name="nobj")
            V.tensor_scalar(nobj[:], fin[0:1, 1:2], EPS, None, OP.add)
            V.reciprocal(nobj[:], nobj[:])
            tot = io.tile([1, 1], f32, name="tot")
            V.tensor_tensor(tot[:], fin[0:1, 0:1], nobj[:], OP.mult)
            nden = io.tile([1, 1], f32, name="nden")
            V.tensor_scalar(nden[:], extras_g[0:1, 1:2], EPS, None, OP.add)
            V.reciprocal(nden[:], nden[:])
            V.tensor_tensor(nden[:], nden[:], extras_g[0:1, 0:1], OP.mult)
            V.tensor_tensor(tot[:], tot[:], nden[:], OP.add)
            lcc = io.tile([1, 1], f32, name="lcc")
            SC.activation(lcc[:], extras_g[0:1, 2:3], AF.Copy,
                          scale=0.001 / (2.0 * N))
            V.tensor_tensor(tot[:], tot[:], lcc[:], OP.add)
            nc.sync.dma_start(out_d.ap(), tot[:])

    nc.compile()
    return nc


def _host_prep(inputs):
    """Slice, pad and re-layout the full inputs into 8 per-core input maps."""
    def lay(a2):                       # [SP, w] -> [128, T, w]
        w = a2.shape[1]
        r = a2.reshape(T, P, w).transpose(1, 0, 2)
        return np.ascontiguousarray(r.astype(np.float32))

    in_maps = []
    for c in range(NCORES):
        sl = slice(c * S, (c + 1) * S)

        def pad(a, fill=0.0):
            out = np.full((SP, a.shape[1]), fill, np.float32)
            out[:S] = a[sl]
            return out

        tidx = np.full((SP, 1), -2.0, np.float32)
        tidx[:S, 0] = inputs["t_idx"][sl, 0].astype(np.float32)
        valid = np.zeros((SP, 1), np.float32)
        valid[:S] = 1.0
        m = {
            "beta_r": lay(pad(inputs["pred_beta"]))[:, :, 0],
            "cc": lay(pad(inputs["pred_ccoords"])),
            "pE": lay(pad(inputs["pred_energy"]))[:, :, 0],
            "ppos": lay(pad(inputs["pred_pos"])),
            "ptime": lay(pad(inputs["pred_time"]))[:, :, 0],
            "pid": lay(pad(inputs["pred_id"])),
            "tE": lay(pad(inputs["t_energy"]))[:, :, 0],
            "tpos": lay(pad(inputs["t_pos"])),
            "ttime": lay(pad(inputs["t_time"]))[:, :, 0],
            "tidx": lay(tidx)[:, :, 0],
            "valid": lay(valid)[:, :, 0],
        }
        m = {k: np.ascontiguousarray(v) for k, v in m.items()}
        in_maps.append(m)
    return in_maps


def _run(inputs, trace=False):
    from concourse import bass_utils
    if "nc" not in _CACHE:
        _CACHE["nc"] = _build()
    nc = _CACHE["nc"]
    in_maps = _host_prep(inputs)
    res = bass_utils.run_bass_kernel_spmd(
        nc, in_maps, core_ids=list(range(NCORES)), trace=trace)
    return res


def kernel(**inputs):
    res = _run(inputs, trace=False)
    val = np.float32(res.results[0]["out"][0, 0])
    return np.array(val, dtype=np.float32)[()]


if __name__ == "__main__":
    d = np.load("/tmp/inputs.npz")
    inp = {k: d[k] for k in d.files}
    print("kernel:", kernel(**inp))
